# revision 19
# baseline (speedup 1.0000x reference)
"""ArcMarginProduct distributed Trainium2 kernel (8 NeuronCores).

Strategy (classifier/tensor parallel along out_features, per sharding hint):
  - weight [100000, 512] is row-sharded across 8 cores: 12500 classes each,
    padded to 12544 = 98*128 rows (pad rows are 1.0, outputs discarded).
  - input [512, 512] and label [512] are replicated (label passed as
    precomputed per-core local index tensors).
  - Each core computes out_i = S * cos(norm(X), norm(W_i)) for its class
    shard, plus the one-hot ArcFace margin applied at (n, label[n]) for the
    labels that fall in its shard (gather W rows -> phi -> scatter fixup).
  - Host concatenates the 8 [512, 12500] column blocks.

Device pipeline per core:
  X: load, row sumsq (ACT Square+accum), rsqrt (DVE recip + ACT sqrt),
     row-scale (f32 + bf16 copies), PE-transpose -> XT (bf16, d on partitions)
  W: stream 98 chunks of [128 rows, 512]: sumsq -> rsqrt -> normalize+cast
     bf16 -> 4x PE-transpose -> drain into per-band WT tiles (bf16)
  MM: per band of 512 classes: 4 n-chunks x 4 k-chunks bf16 matmuls,
      PSUM drain with x30 scale (ACT), DMA out.
  Fixup: indirect-gather W[label] rows, normalize, row-dot vs Xn (f32),
      phi = cos*cos(m) - sin(1-cos^2)*sin(m) with threshold select, x30,
      indirect-scatter 512 values into out (after a barrier).
"""

import math
import sys
import types

import numpy as np

# ---------------- constants (must match reference.py) ----------------
S = 30.0
M = 0.5
COS_M = math.cos(M)
SIN_M = math.sin(M)
TH = math.cos(math.pi - M)
MM = math.sin(math.pi - M) * M

N = 512          # batch
D = 512          # feature dim
C = 100000       # classes
N_CORES = 8
C_PER = C // N_CORES          # 12500
P = 128

_cache = {}


def _ensure_ntff_hook():
    """Install the axon NTFF profiling hook plumbing if this image's antenv
    lacks it (lets run_bass_kernel_spmd(trace=True) return exec_time_ns)."""
    try:
        import antenv.axon_hooks  # noqa: F401
        return
    except ImportError:
        pass
    import antenv
    m = types.ModuleType("antenv.axon_hooks")
    _hook = [None]
    m.set_axon_ntff_profile_hook = lambda h: _hook.__setitem__(0, h)
    m.get_axon_ntff_profile_hook = lambda: _hook[0]
    sys.modules["antenv.axon_hooks"] = m
    antenv.axon_hooks = m
    try:
        from trn_agent_boot.trn_boot import _ntff_profile_via_ctypes
        m.set_axon_ntff_profile_hook(
            _ntff_profile_via_ctypes("/opt/axon/libaxon_pjrt.so"))
    except Exception:
        pass


def build_nc(n_chunks=98, c_per=C_PER, fixup="full"):
    """Build the per-core Bass graph. n_chunks*128 = padded shard width.

    fixup: "full" | "noscatter" | "nogather" | "none"
    """
    from contextlib import ExitStack

    import concourse.bass as bass
    import concourse.tile as tile
    from concourse import bacc, mybir
    from concourse.masks import make_identity

    f32 = mybir.dt.float32
    bf16 = mybir.dt.bfloat16
    i32 = mybir.dt.int32
    A = mybir.AluOpType
    AF = mybir.ActivationFunctionType

    c_pad = n_chunks * P
    n_bands = (n_chunks + 3) // 4        # bands of up to 4 chunks (512 cols)

    nc = bacc.Bacc("TRN2", target_bir_lowering=False, debug=False,
                   num_devices=N_CORES)

    x_d = nc.dram_tensor("x", [N, D], f32, kind="ExternalInput")
    w_d = nc.dram_tensor("w", [c_pad, D], f32, kind="ExternalInput")
    gidx_d = nc.dram_tensor("gidx", [P, 4], i32, kind="ExternalInput")
    sidx_d = nc.dram_tensor("sidx", [P, 4], i32, kind="ExternalInput")
    out_d = nc.dram_tensor("out", [N * c_pad], f32, kind="ExternalOutput")

    out2d = out_d.ap().rearrange("(n c) -> n c", c=c_pad)

    with tile.TileContext(nc) as tc:
        with ExitStack() as ctx:
            const_p = ctx.enter_context(tc.tile_pool(name="const", bufs=1))
            xp = ctx.enter_context(tc.tile_pool(name="xp", bufs=1))
            scr = ctx.enter_context(tc.tile_pool(name="scr", bufs=2))
            wl_p = ctx.enter_context(tc.tile_pool(name="wl", bufs=4))
            wlb_p = ctx.enter_context(tc.tile_pool(name="wlb", bufs=3))
            wnb_p = ctx.enter_context(tc.tile_pool(name="wnb", bufs=6))
            wst_p = ctx.enter_context(tc.tile_pool(name="wst", bufs=4))
            wtb_p = ctx.enter_context(tc.tile_pool(name="wtb", bufs=3))
            ob_p = ctx.enter_context(tc.tile_pool(name="ob", bufs=8))
            fix_p = ctx.enter_context(tc.tile_pool(name="fix", bufs=1))
            ptr_p = ctx.enter_context(
                tc.tile_pool(name="ptr", bufs=3, space="PSUM"))
            pmm_p = ctx.enter_context(
                tc.tile_pool(name="pmm", bufs=5, space="PSUM"))

            ident = const_p.tile([P, P], dtype=bf16)
            make_identity(nc, ident[:])
            ones1 = const_p.tile([1, P], dtype=bf16)
            nc.gpsimd.memset(ones1[:], 1.0)

            # ---------------- X preparation ----------------
            xin = xp.tile([P, 4 * D], dtype=f32)    # chunk g at cols g*512
            xss = xp.tile([P, 4], dtype=f32)
            for g in range(4):
                nc.sync.dma_start(out=xin[:, g * D:(g + 1) * D],
                                  in_=x_d.ap()[g * P:(g + 1) * P, :])
            xsq = scr.tile([P, D], dtype=f32)
            for g in range(4):
                nc.scalar.activation(out=xsq[:], in_=xin[:, g * D:(g + 1) * D],
                                     func=AF.Square,
                                     accum_out=xss[:, g:g + 1])
                xsq = scr.tile([P, D], dtype=f32)
            xrs = xp.tile([P, 4], dtype=f32)      # 1/sumsq
            xrn = xp.tile([P, 4], dtype=f32)      # 1/norm
            nc.vector.reciprocal(out=xrs[:], in_=xss[:])
            nc.scalar.sqrt(out=xrn[:], in_=xrs[:])

            xn = xp.tile([P, 4 * D], dtype=f32)    # normalized X, f32
            xnb = xp.tile([P, 4 * D], dtype=bf16)  # normalized X, bf16
            for g in range(4):
                nc.vector.tensor_scalar_mul(xn[:, g * D:(g + 1) * D],
                                            xin[:, g * D:(g + 1) * D],
                                            xrn[:, g:g + 1])
            nc.vector.tensor_copy(xnb[:], xn[:])

            # XT: [d(part), k-major: k*512 + n] bf16
            xt = xp.tile([P, 4 * N], dtype=bf16)
            for k in range(4):
                pk = ptr_p.tile([P, 4 * P], dtype=bf16, space="PSUM", tag="tp")
                for g in range(4):
                    nc.tensor.transpose(
                        out=pk[:, g * P:(g + 1) * P],
                        in_=xnb[:, g * D + k * P: g * D + (k + 1) * P],
                        identity=ident[:])
                nc.vector.tensor_copy(out=xt[:, k * N:(k + 1) * N], in_=pk[:])

            # ---------------- W stream + matmul ----------------
            # bands of 4 chunks (512 classes); out groups of 4 bands (2048)
            w_chunked = w_d.ap().rearrange("(a p) d -> p a d", p=P)
            n_ogroups = (n_bands + 3) // 4
            ost = {}
            for b in range(n_bands):
                og = b // 4
                chunks = range(b * 4, min((b + 1) * 4, n_chunks))
                nsub = len(chunks)
                ncols = nsub * P

                if b % 4 == 0:
                    # allocate the out-staging tiles for this group
                    gbands = min(4, n_bands - og * 4)
                    gc = sum(
                        len(range(bb * 4, min((bb + 1) * 4, n_chunks))) * P
                        for bb in range(og * 4, og * 4 + gbands))
                    for n in range(4):
                        ost[n] = ob_p.tile([P, 4 * 512], dtype=f32,
                                           tag="ost", name=f"ost{og}_{n}")
                    ost_cols = gc

                # one 4-chunk (up to 1 MiB) W load
                wl = wl_p.tile([P, 4 * D], dtype=f32)
                nc.sync.dma_start(
                    out=wl[:].rearrange("p (a d) -> p a d", d=D)[:, :nsub, :],
                    in_=w_chunked[:, b * 4: b * 4 + nsub, :])
                # raw-cast band to bf16 on GpSimd (norm applied at drain)
                wlb = wlb_p.tile([P, 4 * D], dtype=bf16, tag="wlb")
                nc.gpsimd.tensor_copy(wlb[:, :nsub * D], wl[:, :nsub * D])
                # per-class norms -> S/||w|| broadcast tile for the drain
                wss = wst_p.tile([P, 4], dtype=f32, tag="wss")
                wrs = wst_p.tile([P, 4], dtype=f32, tag="wrs")
                r4b = wst_p.tile([P, 4], dtype=bf16, tag="r4b")
                for s in range(nsub):
                    wsq = scr.tile([P, D], dtype=f32, tag="wsq")
                    nc.scalar.activation(out=wsq[:],
                                         in_=wl[:, s * D:(s + 1) * D],
                                         func=AF.Square,
                                         accum_out=wss[:, s:s + 1])
                nc.vector.reciprocal(out=wrs[:, :nsub], in_=wss[:, :nsub])
                nc.scalar.activation(out=r4b[:, :nsub], in_=wrs[:, :nsub],
                                     func=AF.Sqrt, scale=S * S)
                rt = ptr_p.tile([1, 4 * P], dtype=bf16, space="PSUM",
                                tag="tp")
                for s in range(nsub):
                    nc.tensor.transpose(out=rt[:, s * P:(s + 1) * P],
                                        in_=r4b[:, s:s + 1],
                                        identity=ident[:])
                rsb = wst_p.tile([1, 4 * P], dtype=bf16, tag="rsb")
                nc.vector.tensor_copy(rsb[:, :ncols], rt[:, :ncols])
                bc = ptr_p.tile([P, 512], dtype=f32, space="PSUM", tag="tp")
                for s in range(nsub):
                    nc.tensor.matmul(out=bc[:, s * P:(s + 1) * P],
                                     lhsT=ones1[:],
                                     rhs=rsb[:, s * P:(s + 1) * P],
                                     start=True, stop=True)
                bcs = wst_p.tile([P, 512], dtype=f32, tag="bcs")
                nc.vector.tensor_copy(bcs[:, :ncols], bc[:, :ncols])

                wtb = wtb_p.tile([P, 4 * 512], dtype=bf16)
                for s in range(nsub):
                    wtp = ptr_p.tile([P, 4 * P], dtype=bf16, space="PSUM",
                                     tag="tp")
                    for k in range(4):
                        nc.tensor.transpose(
                            out=wtp[:, k * P:(k + 1) * P],
                            in_=wlb[:, s * D + k * P: s * D + (k + 1) * P],
                            identity=ident[:])
                    # drain psum -> band tile (k-major layout)
                    nc.scalar.copy(
                        out=wtb[:].rearrange("p (k c) -> p k c", k=4)
                            [:, :, s * P:(s + 1) * P],
                        in_=wtp[:].rearrange("p (k c) -> p k c", k=4))

                boff = (b - og * 4) * 512
                for n in range(4):
                    pm = pmm_p.tile([P, 512], dtype=f32, space="PSUM")
                    for k in range(4):
                        nc.tensor.matmul(
                            out=pm[:, :ncols],
                            lhsT=xt[:, k * N + n * P: k * N + (n + 1) * P],
                            rhs=wtb[:, k * 512: k * 512 + ncols],
                            start=(k == 0), stop=(k == 3))
                    nc.vector.tensor_tensor(
                        out=ost[n][:, boff:boff + ncols],
                        in0=pm[:, :ncols], in1=bcs[:, :ncols], op=A.mult)

                if b == n_bands - 1 or b % 4 == 3:
                    # flush the out-staging group (ACT-issued HWDGE)
                    for n in range(4):
                        nc.scalar.dma_start(
                            out=out2d[n * P:(n + 1) * P,
                                      og * 2048: og * 2048 + ost_cols],
                            in_=ost[n][:, :ost_cols])

            # ---------------- sparse margin fixup ----------------
            if fixup != "none":
                gidx = fix_p.tile([P, 4], dtype=i32)
                sidx = fix_p.tile([P, 4], dtype=i32)
                nc.sync.dma_start(out=gidx[:], in_=gidx_d.ap())
                nc.sync.dma_start(out=sidx[:], in_=sidx_d.ap())

                wg = fix_p.tile([P, 4 * D], dtype=f32)
                if fixup != "nogather":
                    for g in range(4):
                        nc.gpsimd.indirect_dma_start(
                            out=wg[:, g * D:(g + 1) * D], out_offset=None,
                            in_=w_d.ap(),
                            in_offset=bass.IndirectOffsetOnAxis(
                                ap=gidx[:, g:g + 1], axis=0))
                else:
                    nc.gpsimd.memset(wg[:], 1.0)

                st = fix_p.tile([P, 16], dtype=f32)   # stat columns
                sumsq = st[:, 0:4]
                for g in range(4):
                    wgsq = scr.tile([P, D], dtype=f32)
                    nc.scalar.activation(out=wgsq[:],
                                         in_=wg[:, g * D:(g + 1) * D],
                                         func=AF.Square,
                                         accum_out=sumsq[:, g:g + 1])
                rs = st[:, 4:8]
                rn = st[:, 8:12]
                nc.vector.reciprocal(out=rs[:], in_=sumsq[:])
                nc.scalar.sqrt(out=rn[:], in_=rs[:])       # 1/||w||

                dots = st[:, 12:16]
                for g in range(4):
                    dsc = scr.tile([P, D], dtype=f32)
                    nc.vector.tensor_tensor(
                        out=dsc[:], in0=xn[:, g * D:(g + 1) * D],
                        in1=wg[:, g * D:(g + 1) * D], op=A.mult)
                    nc.vector.tensor_reduce(
                        out=dots[:, g:g + 1], in_=dsc[:],
                        axis=mybir.AxisListType.X, op=A.add)

                ft = fix_p.tile([P, 4 * 8], dtype=f32)
                cosv, cos2, sine, phi, alt, _unused, vals, tmp = (
                    ft[:, i * 4:(i + 1) * 4] for i in range(8))
                mask_t = fix_p.tile([P, 4], dtype=mybir.dt.uint8)
                mask = mask_t[:]
                nc.vector.tensor_tensor(out=cosv, in0=dots[:], in1=rn[:],
                                        op=A.mult)
                nc.vector.tensor_tensor(out=cos2, in0=cosv, in1=cosv,
                                        op=A.mult)
                nc.vector.tensor_scalar_min(cos2, cos2, 1.0)
                nc.scalar.activation(out=sine, in_=cos2, func=AF.Sqrt,
                                     scale=-1.0, bias=1.0)
                nc.vector.tensor_scalar_mul(phi, cosv, COS_M)
                nc.vector.tensor_scalar_mul(tmp, sine, SIN_M)
                nc.vector.tensor_tensor(out=phi, in0=phi, in1=tmp,
                                        op=A.subtract)
                nc.vector.tensor_scalar_add(alt, cosv, -MM)
                nc.vector.tensor_scalar(out=mask, in0=cosv, scalar1=TH,
                                        scalar2=None, op0=A.is_gt)
                nc.vector.select(out=vals, mask=mask, on_true=phi,
                                 on_false=alt)
                nc.vector.tensor_scalar_mul(vals, vals, S)

                if fixup != "noscatter":
                    # all dense writes must land before the scatter
                    tc.strict_bb_all_engine_barrier()
                    for g in range(4):
                        nc.gpsimd.indirect_dma_start(
                            out=out_d.ap()[:, None],
                            out_offset=bass.IndirectOffsetOnAxis(
                                ap=sidx[:, g:g + 1], axis=0),
                            in_=vals[:, g:g + 1], in_offset=None)
                else:
                    # keep vals live so the ops aren't dead-code eliminated
                    vsink = fix_p.tile([P, 4], dtype=f32)
                    nc.vector.tensor_copy(vsink[:], vals)
                    nc.sync.dma_start(
                        out=out2d[0:P, c_pad - 4:c_pad], in_=vsink[:])

    nc.compile()
    return nc


def make_in_maps(input, label, weight, n_chunks=98, c_per=C_PER):
    """Shard the full inputs into per-core input maps."""
    c_pad = n_chunks * P
    x = np.ascontiguousarray(input, dtype=np.float32)
    lab = np.asarray(label).astype(np.int64)
    w = np.asarray(weight, dtype=np.float32)
    rows = np.arange(N, dtype=np.int64)
    in_maps = []
    for i in range(N_CORES):
        c0 = i * c_per
        wi = np.empty((c_pad, D), dtype=np.float32)
        wi[:c_per] = w[c0:c0 + c_per]
        wi[c_per:] = 1.0
        loc = lab - c0
        valid = (loc >= 0) & (loc < c_per)
        g_rows = np.where(valid, loc, 0).astype(np.int32)
        cols = np.where(valid, loc, np.int64(c_pad - 1))
        s_flat = (rows * c_pad + cols).astype(np.int32)
        in_maps.append({
            "x": x,
            "w": wi,
            "gidx": np.ascontiguousarray(g_rows.reshape(4, P).T),
            "sidx": np.ascontiguousarray(s_flat.reshape(4, P).T),
        })
    return in_maps


def kernel(input, label, weight):
    """Full inputs in, full output out. Runs SPMD on 8 NeuronCores."""
    _ensure_ntff_hook()
    from concourse.bass_utils import run_bass_kernel_spmd

    if "nc" not in _cache:
        _cache["nc"] = build_nc()
    nc = _cache["nc"]

    in_maps = make_in_maps(input, label, weight)
    res = run_bass_kernel_spmd(nc, in_maps, list(range(N_CORES)))
    _cache["last_result"] = res

    c_pad = 98 * P
    out = np.concatenate(
        [res.results[i]["out"].reshape(N, c_pad)[:, :C_PER]
         for i in range(N_CORES)], axis=1)
    return out.astype(np.float32)


# revision 20
# speedup vs baseline: 1.0777x; 1.0777x over previous
"""ArcMarginProduct distributed Trainium2 kernel (8 NeuronCores).

Strategy (classifier/tensor parallel along out_features, per sharding hint):
  - weight [100000, 512] is row-sharded across 8 cores: 12500 classes each,
    padded to 12544 = 98*128 rows (pad rows are 1.0, outputs discarded).
  - input [512, 512] and label [512] are replicated (label passed as
    precomputed per-core local index tensors).
  - Each core computes out_i = S * cos(norm(X), norm(W_i)) for its class
    shard, plus the one-hot ArcFace margin applied at (n, label[n]) for the
    labels that fall in its shard (gather W rows -> phi -> scatter fixup).
  - Host concatenates the 8 [512, 12500] column blocks.

Device pipeline per core:
  X: load, row sumsq (ACT Square+accum), rsqrt (DVE recip + ACT sqrt),
     row-scale (f32 + bf16 copies), PE-transpose -> XT (bf16, d on partitions)
  W: stream 98 chunks of [128 rows, 512]: sumsq -> rsqrt -> normalize+cast
     bf16 -> 4x PE-transpose -> drain into per-band WT tiles (bf16)
  MM: per band of 512 classes: 4 n-chunks x 4 k-chunks bf16 matmuls,
      PSUM drain with x30 scale (ACT), DMA out.
  Fixup: indirect-gather W[label] rows, normalize, row-dot vs Xn (f32),
      phi = cos*cos(m) - sin(1-cos^2)*sin(m) with threshold select, x30,
      indirect-scatter 512 values into out (after a barrier).
"""

import math
import sys
import types

import numpy as np

# ---------------- constants (must match reference.py) ----------------
S = 30.0
M = 0.5
COS_M = math.cos(M)
SIN_M = math.sin(M)
TH = math.cos(math.pi - M)
MM = math.sin(math.pi - M) * M

N = 512          # batch
D = 512          # feature dim
C = 100000       # classes
N_CORES = 8
C_PER = C // N_CORES          # 12500
P = 128

_cache = {}


def _ensure_ntff_hook():
    """Install the axon NTFF profiling hook plumbing if this image's antenv
    lacks it (lets run_bass_kernel_spmd(trace=True) return exec_time_ns)."""
    try:
        import antenv.axon_hooks  # noqa: F401
        return
    except ImportError:
        pass
    import antenv
    m = types.ModuleType("antenv.axon_hooks")
    _hook = [None]
    m.set_axon_ntff_profile_hook = lambda h: _hook.__setitem__(0, h)
    m.get_axon_ntff_profile_hook = lambda: _hook[0]
    sys.modules["antenv.axon_hooks"] = m
    antenv.axon_hooks = m
    try:
        from trn_agent_boot.trn_boot import _ntff_profile_via_ctypes
        m.set_axon_ntff_profile_hook(
            _ntff_profile_via_ctypes("/opt/axon/libaxon_pjrt.so"))
    except Exception:
        pass


def build_nc(n_chunks=98, c_per=C_PER, fixup="full"):
    """Build the per-core Bass graph. n_chunks*128 = padded shard width.

    fixup: "full" | "noscatter" | "nogather" | "none"
    """
    from contextlib import ExitStack

    import concourse.bass as bass
    import concourse.tile as tile
    from concourse import bacc, mybir
    from concourse.masks import make_identity

    f32 = mybir.dt.float32
    bf16 = mybir.dt.bfloat16
    i32 = mybir.dt.int32
    A = mybir.AluOpType
    AF = mybir.ActivationFunctionType

    c_pad = n_chunks * P
    n_bands = (n_chunks + 3) // 4        # bands of up to 4 chunks (512 cols)

    nc = bacc.Bacc("TRN2", target_bir_lowering=False, debug=False,
                   num_devices=N_CORES)

    x_d = nc.dram_tensor("x", [N, D], f32, kind="ExternalInput")
    w_d = nc.dram_tensor("w", [c_pad, D], f32, kind="ExternalInput")
    gidx_d = nc.dram_tensor("gidx", [P, 4], i32, kind="ExternalInput")
    sidx_d = nc.dram_tensor("sidx", [P, 4], i32, kind="ExternalInput")
    out_d = nc.dram_tensor("out", [N * c_pad], f32, kind="ExternalOutput")

    out2d = out_d.ap().rearrange("(n c) -> n c", c=c_pad)

    with tile.TileContext(nc) as tc:
        with ExitStack() as ctx:
            const_p = ctx.enter_context(tc.tile_pool(name="const", bufs=1))
            xp = ctx.enter_context(tc.tile_pool(name="xp", bufs=1))
            scr = ctx.enter_context(tc.tile_pool(name="scr", bufs=2))
            wl_p = ctx.enter_context(tc.tile_pool(name="wl", bufs=4))
            wlb_p = ctx.enter_context(tc.tile_pool(name="wlb", bufs=3))
            wnb_p = ctx.enter_context(tc.tile_pool(name="wnb", bufs=6))
            wst_p = ctx.enter_context(tc.tile_pool(name="wst", bufs=4))
            wtb_p = ctx.enter_context(tc.tile_pool(name="wtb", bufs=3))
            ob_p = ctx.enter_context(tc.tile_pool(name="ob", bufs=8))
            fix_p = ctx.enter_context(tc.tile_pool(name="fix", bufs=1))
            ptr_p = ctx.enter_context(
                tc.tile_pool(name="ptr", bufs=3, space="PSUM"))
            pmm_p = ctx.enter_context(
                tc.tile_pool(name="pmm", bufs=5, space="PSUM"))

            ident = const_p.tile([P, P], dtype=bf16)
            make_identity(nc, ident[:])
            ones1 = const_p.tile([1, P], dtype=bf16)
            nc.gpsimd.memset(ones1[:], 1.0)

            # ---------------- X preparation ----------------
            xin = xp.tile([P, 4 * D], dtype=f32)    # chunk g at cols g*512
            xss = xp.tile([P, 4], dtype=f32)
            for g in range(4):
                nc.sync.dma_start(out=xin[:, g * D:(g + 1) * D],
                                  in_=x_d.ap()[g * P:(g + 1) * P, :])
            xsq = scr.tile([P, D], dtype=f32)
            for g in range(4):
                nc.scalar.activation(out=xsq[:], in_=xin[:, g * D:(g + 1) * D],
                                     func=AF.Square,
                                     accum_out=xss[:, g:g + 1])
                xsq = scr.tile([P, D], dtype=f32)
            xrs = xp.tile([P, 4], dtype=f32)      # 1/sumsq
            xrn = xp.tile([P, 4], dtype=f32)      # 1/norm
            nc.vector.reciprocal(out=xrs[:], in_=xss[:])
            nc.scalar.sqrt(out=xrn[:], in_=xrs[:])

            xn = xp.tile([P, 4 * D], dtype=f32)    # normalized X, f32
            xnb = xp.tile([P, 4 * D], dtype=bf16)  # normalized X, bf16
            for g in range(4):
                nc.vector.tensor_scalar_mul(xn[:, g * D:(g + 1) * D],
                                            xin[:, g * D:(g + 1) * D],
                                            xrn[:, g:g + 1])
            nc.vector.tensor_copy(xnb[:], xn[:])

            # XT: [d(part), k-major: k*512 + n] bf16
            xt = xp.tile([P, 4 * N], dtype=bf16)
            for k in range(4):
                pk = ptr_p.tile([P, 4 * P], dtype=bf16, space="PSUM", tag="tp")
                for g in range(4):
                    nc.tensor.transpose(
                        out=pk[:, g * P:(g + 1) * P],
                        in_=xnb[:, g * D + k * P: g * D + (k + 1) * P],
                        identity=ident[:])
                nc.vector.tensor_copy(out=xt[:, k * N:(k + 1) * N], in_=pk[:])

            # ---------------- W stream + matmul ----------------
            # bands of 4 chunks (512 classes); out groups of 4 bands (2048)
            w_chunked = w_d.ap().rearrange("(a p) d -> p a d", p=P)
            n_ogroups = (n_bands + 3) // 4
            ost = {}
            for b in range(n_bands):
                og = b // 4
                chunks = range(b * 4, min((b + 1) * 4, n_chunks))
                nsub = len(chunks)
                ncols = nsub * P

                if b % 4 == 0:
                    # allocate the out-staging tiles for this group
                    gbands = min(4, n_bands - og * 4)
                    gc = sum(
                        len(range(bb * 4, min((bb + 1) * 4, n_chunks))) * P
                        for bb in range(og * 4, og * 4 + gbands))
                    for n in range(4):
                        ost[n] = ob_p.tile([P, 4 * 512], dtype=f32,
                                           tag="ost", name=f"ost{og}_{n}")
                    ost_cols = gc

                # one 4-chunk (up to 1 MiB) W load
                wl = wl_p.tile([P, 4 * D], dtype=f32)
                nc.sync.dma_start(
                    out=wl[:].rearrange("p (a d) -> p a d", d=D)[:, :nsub, :],
                    in_=w_chunked[:, b * 4: b * 4 + nsub, :])
                # per-class norms
                wss = wst_p.tile([P, 4], dtype=f32, tag="wss")
                wrs = wst_p.tile([P, 4], dtype=f32, tag="wrs")
                wrn = wst_p.tile([P, 4], dtype=f32, tag="wrn")
                for s in range(nsub):
                    wsq = scr.tile([P, D], dtype=f32, tag="wsq")
                    nc.scalar.activation(out=wsq[:],
                                         in_=wl[:, s * D:(s + 1) * D],
                                         func=AF.Square,
                                         accum_out=wss[:, s:s + 1])
                nc.vector.reciprocal(out=wrs[:, :nsub], in_=wss[:, :nsub])
                nc.scalar.activation(out=wrn[:, :nsub], in_=wrs[:, :nsub],
                                     func=AF.Sqrt)

                wtb = wtb_p.tile([P, 4 * 512], dtype=bf16)
                for s in range(nsub):
                    # fused normalize + cast to bf16
                    wnb = wnb_p.tile([P, D], dtype=bf16, tag="wnb")
                    nc.vector.tensor_scalar_mul(wnb[:],
                                                wl[:, s * D:(s + 1) * D],
                                                wrn[:, s:s + 1])
                    wtp = ptr_p.tile([P, 4 * P], dtype=bf16, space="PSUM",
                                     tag="tp")
                    for k in range(4):
                        nc.tensor.transpose(
                            out=wtp[:, k * P:(k + 1) * P],
                            in_=wnb[:, k * P:(k + 1) * P],
                            identity=ident[:])
                    # drain psum -> band tile (k-major layout)
                    nc.vector.tensor_copy(
                        out=wtb[:].rearrange("p (k c) -> p k c", k=4)
                            [:, :, s * P:(s + 1) * P],
                        in_=wtp[:].rearrange("p (k c) -> p k c", k=4))

                boff = (b - og * 4) * 512
                for n in range(4):
                    pm = pmm_p.tile([P, 512], dtype=f32, space="PSUM")
                    for k in range(4):
                        nc.tensor.matmul(
                            out=pm[:, :ncols],
                            lhsT=xt[:, k * N + n * P: k * N + (n + 1) * P],
                            rhs=wtb[:, k * 512: k * 512 + ncols],
                            start=(k == 0), stop=(k == 3))
                    if n < 3:
                        nc.scalar.mul(out=ost[n][:, boff:boff + ncols],
                                      in_=pm[:, :ncols], mul=S)
                    else:
                        nc.vector.tensor_scalar_mul(
                            ost[n][:, boff:boff + ncols], pm[:, :ncols], S)

                if b == n_bands - 1 or b % 4 == 3:
                    # flush the out-staging group (ACT-issued HWDGE)
                    for n in range(4):
                        nc.scalar.dma_start(
                            out=out2d[n * P:(n + 1) * P,
                                      og * 2048: og * 2048 + ost_cols],
                            in_=ost[n][:, :ost_cols])

            # ---------------- sparse margin fixup ----------------
            if fixup != "none":
                gidx = fix_p.tile([P, 4], dtype=i32)
                sidx = fix_p.tile([P, 4], dtype=i32)
                nc.sync.dma_start(out=gidx[:], in_=gidx_d.ap())
                nc.sync.dma_start(out=sidx[:], in_=sidx_d.ap())

                wg = fix_p.tile([P, 4 * D], dtype=f32)
                if fixup != "nogather":
                    for g in range(4):
                        nc.gpsimd.indirect_dma_start(
                            out=wg[:, g * D:(g + 1) * D], out_offset=None,
                            in_=w_d.ap(),
                            in_offset=bass.IndirectOffsetOnAxis(
                                ap=gidx[:, g:g + 1], axis=0))
                else:
                    nc.gpsimd.memset(wg[:], 1.0)

                st = fix_p.tile([P, 16], dtype=f32)   # stat columns
                sumsq = st[:, 0:4]
                for g in range(4):
                    wgsq = scr.tile([P, D], dtype=f32)
                    nc.scalar.activation(out=wgsq[:],
                                         in_=wg[:, g * D:(g + 1) * D],
                                         func=AF.Square,
                                         accum_out=sumsq[:, g:g + 1])
                rs = st[:, 4:8]
                rn = st[:, 8:12]
                nc.vector.reciprocal(out=rs[:], in_=sumsq[:])
                nc.scalar.sqrt(out=rn[:], in_=rs[:])       # 1/||w||

                dots = st[:, 12:16]
                for g in range(4):
                    dsc = scr.tile([P, D], dtype=f32)
                    nc.vector.tensor_tensor(
                        out=dsc[:], in0=xn[:, g * D:(g + 1) * D],
                        in1=wg[:, g * D:(g + 1) * D], op=A.mult)
                    nc.vector.tensor_reduce(
                        out=dots[:, g:g + 1], in_=dsc[:],
                        axis=mybir.AxisListType.X, op=A.add)

                ft = fix_p.tile([P, 4 * 8], dtype=f32)
                cosv, cos2, sine, phi, alt, _unused, vals, tmp = (
                    ft[:, i * 4:(i + 1) * 4] for i in range(8))
                mask_t = fix_p.tile([P, 4], dtype=mybir.dt.uint8)
                mask = mask_t[:]
                nc.vector.tensor_tensor(out=cosv, in0=dots[:], in1=rn[:],
                                        op=A.mult)
                nc.vector.tensor_tensor(out=cos2, in0=cosv, in1=cosv,
                                        op=A.mult)
                nc.vector.tensor_scalar_min(cos2, cos2, 1.0)
                nc.scalar.activation(out=sine, in_=cos2, func=AF.Sqrt,
                                     scale=-1.0, bias=1.0)
                nc.vector.tensor_scalar_mul(phi, cosv, COS_M)
                nc.vector.tensor_scalar_mul(tmp, sine, SIN_M)
                nc.vector.tensor_tensor(out=phi, in0=phi, in1=tmp,
                                        op=A.subtract)
                nc.vector.tensor_scalar_add(alt, cosv, -MM)
                nc.vector.tensor_scalar(out=mask, in0=cosv, scalar1=TH,
                                        scalar2=None, op0=A.is_gt)
                nc.vector.select(out=vals, mask=mask, on_true=phi,
                                 on_false=alt)
                nc.vector.tensor_scalar_mul(vals, vals, S)

                if fixup != "noscatter":
                    # all dense writes must land before the scatter
                    tc.strict_bb_all_engine_barrier()
                    for g in range(4):
                        nc.gpsimd.indirect_dma_start(
                            out=out_d.ap()[:, None],
                            out_offset=bass.IndirectOffsetOnAxis(
                                ap=sidx[:, g:g + 1], axis=0),
                            in_=vals[:, g:g + 1], in_offset=None)
                else:
                    # keep vals live so the ops aren't dead-code eliminated
                    vsink = fix_p.tile([P, 4], dtype=f32)
                    nc.vector.tensor_copy(vsink[:], vals)
                    nc.sync.dma_start(
                        out=out2d[0:P, c_pad - 4:c_pad], in_=vsink[:])

    nc.compile()
    return nc


def make_in_maps(input, label, weight, n_chunks=98, c_per=C_PER):
    """Shard the full inputs into per-core input maps."""
    c_pad = n_chunks * P
    x = np.ascontiguousarray(input, dtype=np.float32)
    lab = np.asarray(label).astype(np.int64)
    w = np.asarray(weight, dtype=np.float32)
    rows = np.arange(N, dtype=np.int64)
    in_maps = []
    for i in range(N_CORES):
        c0 = i * c_per
        wi = np.empty((c_pad, D), dtype=np.float32)
        wi[:c_per] = w[c0:c0 + c_per]
        wi[c_per:] = 1.0
        loc = lab - c0
        valid = (loc >= 0) & (loc < c_per)
        g_rows = np.where(valid, loc, 0).astype(np.int32)
        cols = np.where(valid, loc, np.int64(c_pad - 1))
        s_flat = (rows * c_pad + cols).astype(np.int32)
        in_maps.append({
            "x": x,
            "w": wi,
            "gidx": np.ascontiguousarray(g_rows.reshape(4, P).T),
            "sidx": np.ascontiguousarray(s_flat.reshape(4, P).T),
        })
    return in_maps


def kernel(input, label, weight):
    """Full inputs in, full output out. Runs SPMD on 8 NeuronCores."""
    _ensure_ntff_hook()
    from concourse.bass_utils import run_bass_kernel_spmd

    if "nc" not in _cache:
        _cache["nc"] = build_nc()
    nc = _cache["nc"]

    in_maps = make_in_maps(input, label, weight)
    res = run_bass_kernel_spmd(nc, in_maps, list(range(N_CORES)))
    _cache["last_result"] = res

    c_pad = 98 * P
    out = np.concatenate(
        [res.results[i]["out"].reshape(N, c_pad)[:, :C_PER]
         for i in range(N_CORES)], axis=1)
    return out.astype(np.float32)


# revision 26
# speedup vs baseline: 1.0992x; 1.0199x over previous
"""ArcMarginProduct distributed Trainium2 kernel (8 NeuronCores).

Strategy (classifier/tensor parallel along out_features, per sharding hint):
  - weight [100000, 512] is row-sharded across 8 cores: 12500 classes each,
    padded to 12544 = 98*128 rows (pad rows are 1.0, outputs discarded).
  - input [512, 512] and label [512] are replicated (label passed as
    precomputed per-core local index tensors).
  - Each core computes out_i = S * cos(norm(X), norm(W_i)) for its class
    shard, plus the one-hot ArcFace margin applied at (n, label[n]) for the
    labels that fall in its shard (gather W rows -> phi -> scatter fixup).
  - Host concatenates the 8 [512, 12500] column blocks.

Device pipeline per core:
  X: load, row sumsq (ACT Square+accum), rsqrt (DVE recip + ACT sqrt),
     row-scale (f32 + bf16 copies), PE-transpose -> XT (bf16, d on partitions)
  W: stream 98 chunks of [128 rows, 512]: sumsq -> rsqrt -> normalize+cast
     bf16 -> 4x PE-transpose -> drain into per-band WT tiles (bf16)
  MM: per band of 512 classes: 4 n-chunks x 4 k-chunks bf16 matmuls,
      PSUM drain with x30 scale (ACT), DMA out.
  Fixup: indirect-gather W[label] rows, normalize, row-dot vs Xn (f32),
      phi = cos*cos(m) - sin(1-cos^2)*sin(m) with threshold select, x30,
      indirect-scatter 512 values into out (after a barrier).
"""

import math
import sys
import types

import numpy as np

# ---------------- constants (must match reference.py) ----------------
S = 30.0
M = 0.5
COS_M = math.cos(M)
SIN_M = math.sin(M)
TH = math.cos(math.pi - M)
MM = math.sin(math.pi - M) * M

N = 512          # batch
D = 512          # feature dim
C = 100000       # classes
N_CORES = 8
C_PER = C // N_CORES          # 12500
P = 128

_cache = {}


def _ensure_ntff_hook():
    """Install the axon NTFF profiling hook plumbing if this image's antenv
    lacks it (lets run_bass_kernel_spmd(trace=True) return exec_time_ns)."""
    try:
        import antenv.axon_hooks  # noqa: F401
        return
    except ImportError:
        pass
    import antenv
    m = types.ModuleType("antenv.axon_hooks")
    _hook = [None]
    m.set_axon_ntff_profile_hook = lambda h: _hook.__setitem__(0, h)
    m.get_axon_ntff_profile_hook = lambda: _hook[0]
    sys.modules["antenv.axon_hooks"] = m
    antenv.axon_hooks = m
    try:
        from trn_agent_boot.trn_boot import _ntff_profile_via_ctypes
        m.set_axon_ntff_profile_hook(
            _ntff_profile_via_ctypes("/opt/axon/libaxon_pjrt.so"))
    except Exception:
        pass


def build_nc(n_chunks=98, c_per=C_PER, fixup="full", out_bf16=True):
    """Build the per-core Bass graph. n_chunks*128 = padded shard width.

    fixup: "full" | "noscatter" | "nogather" | "none"
    """
    from contextlib import ExitStack

    import concourse.bass as bass
    import concourse.tile as tile
    from concourse import bacc, mybir
    from concourse.masks import make_identity

    f32 = mybir.dt.float32
    bf16 = mybir.dt.bfloat16
    i32 = mybir.dt.int32
    A = mybir.AluOpType
    AF = mybir.ActivationFunctionType

    c_pad = n_chunks * P
    n_bands = (n_chunks + 3) // 4        # bands of up to 4 chunks (512 cols)

    nc = bacc.Bacc("TRN2", target_bir_lowering=False, debug=False,
                   num_devices=N_CORES)

    x_d = nc.dram_tensor("x", [N, D], f32, kind="ExternalInput")
    w_d = nc.dram_tensor("w", [c_pad, D], f32, kind="ExternalInput")
    gidx_d = nc.dram_tensor("gidx", [P, 4], i32, kind="ExternalInput")
    sidx_d = nc.dram_tensor("sidx", [P, 4], i32, kind="ExternalInput")
    out_dt = bf16 if out_bf16 else f32
    out_d = nc.dram_tensor("out", [N * c_pad], out_dt, kind="ExternalOutput")

    out2d = out_d.ap().rearrange("(n c) -> n c", c=c_pad)

    with tile.TileContext(nc) as tc:
        with ExitStack() as ctx:
            const_p = ctx.enter_context(tc.tile_pool(name="const", bufs=1))
            xp = ctx.enter_context(tc.tile_pool(name="xp", bufs=1))
            scr = ctx.enter_context(tc.tile_pool(name="scr", bufs=2))
            wl_p = ctx.enter_context(tc.tile_pool(name="wl", bufs=4))
            wlb_p = ctx.enter_context(tc.tile_pool(name="wlb", bufs=3))
            wnb_p = ctx.enter_context(tc.tile_pool(name="wnb", bufs=6))
            wst_p = ctx.enter_context(tc.tile_pool(name="wst", bufs=4))
            wtb_p = ctx.enter_context(tc.tile_pool(name="wtb", bufs=3))
            ob_p = ctx.enter_context(tc.tile_pool(name="ob", bufs=8))
            fix_p = ctx.enter_context(tc.tile_pool(name="fix", bufs=1))
            ptr_p = ctx.enter_context(
                tc.tile_pool(name="ptr", bufs=3, space="PSUM"))
            pmm_p = ctx.enter_context(
                tc.tile_pool(name="pmm", bufs=5, space="PSUM"))

            ident = const_p.tile([P, P], dtype=bf16)
            make_identity(nc, ident[:])
            ones1 = const_p.tile([1, P], dtype=bf16)
            nc.gpsimd.memset(ones1[:], 1.0)

            # ---------------- X preparation ----------------
            xin = xp.tile([P, 4 * D], dtype=f32)    # chunk g at cols g*512
            xss = xp.tile([P, 4], dtype=f32)
            for g in range(4):
                nc.sync.dma_start(out=xin[:, g * D:(g + 1) * D],
                                  in_=x_d.ap()[g * P:(g + 1) * P, :])
            xsq = scr.tile([P, D], dtype=f32)
            for g in range(4):
                nc.scalar.activation(out=xsq[:], in_=xin[:, g * D:(g + 1) * D],
                                     func=AF.Square,
                                     accum_out=xss[:, g:g + 1])
                xsq = scr.tile([P, D], dtype=f32)
            xrs = xp.tile([P, 4], dtype=f32)      # 1/sumsq
            xrn = xp.tile([P, 4], dtype=f32)      # 1/norm
            nc.vector.reciprocal(out=xrs[:], in_=xss[:])
            nc.scalar.sqrt(out=xrn[:], in_=xrs[:])

            xn = xp.tile([P, 4 * D], dtype=f32)    # normalized X, f32
            xnb = xp.tile([P, 4 * D], dtype=bf16)  # normalized X, bf16
            for g in range(4):
                nc.vector.tensor_scalar_mul(xn[:, g * D:(g + 1) * D],
                                            xin[:, g * D:(g + 1) * D],
                                            xrn[:, g:g + 1])
            nc.vector.tensor_copy(xnb[:], xn[:])

            # XT: [d(part), k-major: k*512 + n] bf16
            xt = xp.tile([P, 4 * N], dtype=bf16)
            for k in range(4):
                pk = ptr_p.tile([P, 4 * P], dtype=bf16, space="PSUM", tag="tp")
                for g in range(4):
                    nc.tensor.transpose(
                        out=pk[:, g * P:(g + 1) * P],
                        in_=xnb[:, g * D + k * P: g * D + (k + 1) * P],
                        identity=ident[:])
                nc.vector.tensor_copy(out=xt[:, k * N:(k + 1) * N], in_=pk[:])

            # ---------------- sparse margin fixup (compute early) --------
            vals = None
            if fixup != "none":
                gidx = fix_p.tile([P, 4], dtype=i32)
                sidx = fix_p.tile([P, 4], dtype=i32)
                nc.sync.dma_start(out=gidx[:], in_=gidx_d.ap())
                nc.sync.dma_start(out=sidx[:], in_=sidx_d.ap())

                wg = fix_p.tile([P, 4 * D], dtype=f32)
                if fixup != "nogather":
                    for g in range(4):
                        nc.gpsimd.indirect_dma_start(
                            out=wg[:, g * D:(g + 1) * D], out_offset=None,
                            in_=w_d.ap(),
                            in_offset=bass.IndirectOffsetOnAxis(
                                ap=gidx[:, g:g + 1], axis=0))
                else:
                    nc.gpsimd.memset(wg[:], 1.0)

                st = fix_p.tile([P, 16], dtype=f32)   # stat columns
                sumsq = st[:, 0:4]
                for g in range(4):
                    wgsq = scr.tile([P, D], dtype=f32)
                    nc.scalar.activation(out=wgsq[:],
                                         in_=wg[:, g * D:(g + 1) * D],
                                         func=AF.Square,
                                         accum_out=sumsq[:, g:g + 1])
                rs = st[:, 4:8]
                rn = st[:, 8:12]
                nc.vector.reciprocal(out=rs[:], in_=sumsq[:])
                nc.scalar.sqrt(out=rn[:], in_=rs[:])       # 1/||w||

                dots = st[:, 12:16]
                for g in range(4):
                    dsc = scr.tile([P, D], dtype=f32)
                    nc.vector.tensor_tensor(
                        out=dsc[:], in0=xn[:, g * D:(g + 1) * D],
                        in1=wg[:, g * D:(g + 1) * D], op=A.mult)
                    nc.vector.tensor_reduce(
                        out=dots[:, g:g + 1], in_=dsc[:],
                        axis=mybir.AxisListType.X, op=A.add)

                ft = fix_p.tile([P, 4 * 8], dtype=f32)
                cosv, cos2, sine, phi, alt, _unused, fvals, tmp = (
                    ft[:, i * 4:(i + 1) * 4] for i in range(8))
                mask_t = fix_p.tile([P, 4], dtype=mybir.dt.uint8)
                mask = mask_t[:]
                nc.vector.tensor_tensor(out=cosv, in0=dots[:], in1=rn[:],
                                        op=A.mult)
                nc.vector.tensor_tensor(out=cos2, in0=cosv, in1=cosv,
                                        op=A.mult)
                nc.vector.tensor_scalar_min(cos2, cos2, 1.0)
                nc.scalar.activation(out=sine, in_=cos2, func=AF.Sqrt,
                                     scale=-1.0, bias=1.0)
                nc.vector.tensor_scalar_mul(phi, cosv, COS_M)
                nc.vector.tensor_scalar_mul(tmp, sine, SIN_M)
                nc.vector.tensor_tensor(out=phi, in0=phi, in1=tmp,
                                        op=A.subtract)
                nc.vector.tensor_scalar_add(alt, cosv, -MM)
                nc.vector.tensor_scalar(out=mask, in0=cosv, scalar1=TH,
                                        scalar2=None, op0=A.is_gt)
                nc.vector.select(out=fvals, mask=mask, on_true=phi,
                                 on_false=alt)
                nc.vector.tensor_scalar_mul(fvals, fvals, S)
                if out_bf16:
                    vals_t = fix_p.tile([P, 4], dtype=bf16)
                    nc.vector.tensor_copy(vals_t[:], fvals)
                    vals = vals_t[:]
                else:
                    vals = fvals

            # ---------------- W stream + matmul ----------------
            # bands of 4 chunks (512 classes); out groups of 4 bands (2048)
            w_chunked = w_d.ap().rearrange("(a p) d -> p a d", p=P)
            n_ogroups = (n_bands + 3) // 4
            ost = {}
            for b in range(n_bands):
                og = b // 4
                chunks = range(b * 4, min((b + 1) * 4, n_chunks))
                nsub = len(chunks)
                ncols = nsub * P

                if b % 4 == 0:
                    # allocate the out-staging tiles for this group
                    gbands = min(4, n_bands - og * 4)
                    gc = sum(
                        len(range(bb * 4, min((bb + 1) * 4, n_chunks))) * P
                        for bb in range(og * 4, og * 4 + gbands))
                    for n in range(4):
                        ost[n] = ob_p.tile([P, 4 * 512], dtype=out_dt,
                                           tag="ost", name=f"ost{og}_{n}")
                    ost_cols = gc

                # one 4-chunk (up to 1 MiB) W load
                wl = wl_p.tile([P, 4 * D], dtype=f32)
                nc.sync.dma_start(
                    out=wl[:].rearrange("p (a d) -> p a d", d=D)[:, :nsub, :],
                    in_=w_chunked[:, b * 4: b * 4 + nsub, :])
                # per-class norms
                wss = wst_p.tile([P, 4], dtype=f32, tag="wss")
                wrs = wst_p.tile([P, 4], dtype=f32, tag="wrs")
                wrn = wst_p.tile([P, 4], dtype=f32, tag="wrn")
                for s in range(nsub):
                    wsq = scr.tile([P, D], dtype=f32, tag="wsq")
                    nc.scalar.activation(out=wsq[:],
                                         in_=wl[:, s * D:(s + 1) * D],
                                         func=AF.Square,
                                         accum_out=wss[:, s:s + 1])
                nc.vector.reciprocal(out=wrs[:, :nsub], in_=wss[:, :nsub])
                nc.scalar.activation(out=wrn[:, :nsub], in_=wrs[:, :nsub],
                                     func=AF.Sqrt)

                wtb = wtb_p.tile([P, 4 * 512], dtype=bf16)
                for s0 in range(0, nsub, 2):
                    # two chunks per psum tile -> one batched drain
                    wtp = ptr_p.tile([P, 8 * P], dtype=bf16, space="PSUM",
                                     tag="tp")
                    for ds in range(2):
                        s = s0 + ds
                        # fused normalize + cast to bf16
                        wnb = wnb_p.tile([P, D], dtype=bf16, tag="wnb")
                        nc.vector.tensor_scalar_mul(wnb[:],
                                                    wl[:, s * D:(s + 1) * D],
                                                    wrn[:, s:s + 1])
                        for k in range(4):
                            nc.tensor.transpose(
                                out=wtp[:, k * 2 * P + ds * P:
                                        k * 2 * P + (ds + 1) * P],
                                in_=wnb[:, k * P:(k + 1) * P],
                                identity=ident[:])
                    # drain psum -> band tile (k-major layout)
                    nc.vector.tensor_copy(
                        out=wtb[:].rearrange("p (k c) -> p k c", k=4)
                            [:, :, s0 * P:(s0 + 2) * P],
                        in_=wtp[:].rearrange("p (k c) -> p k c", k=4))

                boff = (b - og * 4) * 512
                for n in range(4):
                    pm = pmm_p.tile([P, 512], dtype=f32, space="PSUM")
                    for k in range(4):
                        nc.tensor.matmul(
                            out=pm[:, :ncols],
                            lhsT=xt[:, k * N + n * P: k * N + (n + 1) * P],
                            rhs=wtb[:, k * 512: k * 512 + ncols],
                            start=(k == 0), stop=(k == 3))
                    if n < 3:
                        nc.scalar.mul(out=ost[n][:, boff:boff + ncols],
                                      in_=pm[:, :ncols], mul=S)
                    else:
                        nc.vector.tensor_scalar_mul(
                            ost[n][:, boff:boff + ncols], pm[:, :ncols], S)

                if b == n_bands - 1 or b % 4 == 3:
                    # flush the out-staging group (ACT-issued HWDGE)
                    for n in range(4):
                        nc.scalar.dma_start(
                            out=out2d[n * P:(n + 1) * P,
                                      og * 2048: og * 2048 + ost_cols],
                            in_=ost[n][:, :ost_cols])

            # ---------------- scatter the margin values ----------------
            if fixup not in ("none", "noscatter"):
                # all dense writes must land before the scatter
                tc.strict_bb_all_engine_barrier()
                for g in range(4):
                    nc.gpsimd.indirect_dma_start(
                        out=out_d.ap()[:, None],
                        out_offset=bass.IndirectOffsetOnAxis(
                            ap=sidx[:, g:g + 1], axis=0),
                        in_=vals[:, g:g + 1], in_offset=None)

    nc.compile()
    return nc


def make_in_maps(input, label, weight, n_chunks=98, c_per=C_PER):
    """Shard the full inputs into per-core input maps."""
    c_pad = n_chunks * P
    x = np.ascontiguousarray(input, dtype=np.float32)
    lab = np.asarray(label).astype(np.int64)
    w = np.asarray(weight, dtype=np.float32)
    rows = np.arange(N, dtype=np.int64)
    in_maps = []
    for i in range(N_CORES):
        c0 = i * c_per
        wi = np.empty((c_pad, D), dtype=np.float32)
        wi[:c_per] = w[c0:c0 + c_per]
        wi[c_per:] = 1.0
        loc = lab - c0
        valid = (loc >= 0) & (loc < c_per)
        g_rows = np.where(valid, loc, 0).astype(np.int32)
        cols = np.where(valid, loc, np.int64(c_pad - 1))
        s_flat = (rows * c_pad + cols).astype(np.int32)
        in_maps.append({
            "x": x,
            "w": wi,
            "gidx": np.ascontiguousarray(g_rows.reshape(4, P).T),
            "sidx": np.ascontiguousarray(s_flat.reshape(4, P).T),
        })
    return in_maps


def kernel(input, label, weight):
    """Full inputs in, full output out. Runs SPMD on 8 NeuronCores."""
    _ensure_ntff_hook()
    from concourse.bass_utils import run_bass_kernel_spmd

    if "nc" not in _cache:
        _cache["nc"] = build_nc()
    nc = _cache["nc"]

    in_maps = make_in_maps(input, label, weight)
    res = run_bass_kernel_spmd(nc, in_maps, list(range(N_CORES)))
    _cache["last_result"] = res

    c_pad = 98 * P
    out = np.concatenate(
        [res.results[i]["out"].reshape(N, c_pad)[:, :C_PER]
         for i in range(N_CORES)], axis=1)
    return out.astype(np.float32)


# revision 28
# speedup vs baseline: 1.2975x; 1.1805x over previous
"""ArcMarginProduct distributed Trainium2 kernel (8 NeuronCores).

Strategy (classifier/tensor parallel along out_features, per sharding hint):
  - weight [100000, 512] is row-sharded across 8 cores: 12500 classes each,
    padded to 12544 = 98*128 rows (pad rows are 1.0, outputs discarded).
  - input [512, 512] and label [512] are replicated (label passed as
    precomputed per-core local index tensors).
  - Each core computes out_i = S * cos(norm(X), norm(W_i)) for its class
    shard, plus the one-hot ArcFace margin applied at (n, label[n]) for the
    labels that fall in its shard (gather W rows -> phi -> scatter fixup).
  - Host concatenates the 8 [512, 12500] column blocks.

Device pipeline per core:
  X: load, row sumsq (ACT Square+accum), rsqrt (DVE recip + ACT sqrt),
     row-scale (f32 + bf16 copies), PE-transpose -> XT (bf16, d on partitions)
  W: stream 98 chunks of [128 rows, 512]: sumsq -> rsqrt -> normalize+cast
     bf16 -> 4x PE-transpose -> drain into per-band WT tiles (bf16)
  MM: per band of 512 classes: 4 n-chunks x 4 k-chunks bf16 matmuls,
      PSUM drain with x30 scale (ACT), DMA out.
  Fixup: indirect-gather W[label] rows, normalize, row-dot vs Xn (f32),
      phi = cos*cos(m) - sin(1-cos^2)*sin(m) with threshold select, x30,
      indirect-scatter 512 values into out (after a barrier).
"""

import math
import sys
import types

import numpy as np

# ---------------- constants (must match reference.py) ----------------
S = 30.0
M = 0.5
COS_M = math.cos(M)
SIN_M = math.sin(M)
TH = math.cos(math.pi - M)
MM = math.sin(math.pi - M) * M

N = 512          # batch
D = 512          # feature dim
C = 100000       # classes
N_CORES = 8
C_PER = C // N_CORES          # 12500
P = 128

_cache = {}


def _ensure_ntff_hook():
    """Install the axon NTFF profiling hook plumbing if this image's antenv
    lacks it (lets run_bass_kernel_spmd(trace=True) return exec_time_ns)."""
    try:
        import antenv.axon_hooks  # noqa: F401
        return
    except ImportError:
        pass
    import antenv
    m = types.ModuleType("antenv.axon_hooks")
    _hook = [None]
    m.set_axon_ntff_profile_hook = lambda h: _hook.__setitem__(0, h)
    m.get_axon_ntff_profile_hook = lambda: _hook[0]
    sys.modules["antenv.axon_hooks"] = m
    antenv.axon_hooks = m
    try:
        from trn_agent_boot.trn_boot import _ntff_profile_via_ctypes
        m.set_axon_ntff_profile_hook(
            _ntff_profile_via_ctypes("/opt/axon/libaxon_pjrt.so"))
    except Exception:
        pass


def build_nc(n_chunks=98, c_per=C_PER, fixup="full", out_bf16=True):
    """Build the per-core Bass graph. n_chunks*128 = padded shard width.

    fixup: "full" | "noscatter" | "nogather" | "none"
    """
    from contextlib import ExitStack

    import concourse.bass as bass
    import concourse.tile as tile
    from concourse import bacc, mybir
    from concourse.masks import make_identity

    f32 = mybir.dt.float32
    bf16 = mybir.dt.bfloat16
    i32 = mybir.dt.int32
    A = mybir.AluOpType
    AF = mybir.ActivationFunctionType

    c_pad = n_chunks * P
    n_bands = (n_chunks + 3) // 4        # bands of up to 4 chunks (512 cols)

    nc = bacc.Bacc("TRN2", target_bir_lowering=False, debug=False,
                   num_devices=N_CORES)

    x_d = nc.dram_tensor("x", [N, D], f32, kind="ExternalInput")
    w_d = nc.dram_tensor("w", [c_pad, D], f32, kind="ExternalInput")
    gidx_d = nc.dram_tensor("gidx", [P, 4], i32, kind="ExternalInput")
    sidx_d = nc.dram_tensor("sidx", [P, 4], i32, kind="ExternalInput")
    out_dt = bf16 if out_bf16 else f32
    out_d = nc.dram_tensor("out", [N * c_pad], out_dt, kind="ExternalOutput")

    out2d = out_d.ap().rearrange("(n c) -> n c", c=c_pad)

    with tile.TileContext(nc) as tc:
        with ExitStack() as ctx:
            const_p = ctx.enter_context(tc.tile_pool(name="const", bufs=1))
            xp = ctx.enter_context(tc.tile_pool(name="xp", bufs=1))
            scr = ctx.enter_context(tc.tile_pool(name="scr", bufs=2))
            wl_p = ctx.enter_context(tc.tile_pool(name="wl", bufs=4))
            wlb_p = ctx.enter_context(tc.tile_pool(name="wlb", bufs=3))
            wnb_p = ctx.enter_context(tc.tile_pool(name="wnb", bufs=6))
            wst_p = ctx.enter_context(tc.tile_pool(name="wst", bufs=4))
            wtb_p = ctx.enter_context(tc.tile_pool(name="wtb", bufs=3))
            ob_p = ctx.enter_context(tc.tile_pool(name="ob", bufs=12))
            fix_p = ctx.enter_context(tc.tile_pool(name="fix", bufs=1))
            ptr_p = ctx.enter_context(
                tc.tile_pool(name="ptr", bufs=3, space="PSUM"))
            pmm_p = ctx.enter_context(
                tc.tile_pool(name="pmm", bufs=5, space="PSUM"))

            ident = const_p.tile([P, P], dtype=bf16)
            make_identity(nc, ident[:])
            ones1 = const_p.tile([1, P], dtype=bf16)
            nc.gpsimd.memset(ones1[:], 1.0)

            # ---------------- X preparation ----------------
            xin = xp.tile([P, 4 * D], dtype=f32)    # chunk g at cols g*512
            xss = xp.tile([P, 4], dtype=f32)
            for g in range(4):
                nc.sync.dma_start(out=xin[:, g * D:(g + 1) * D],
                                  in_=x_d.ap()[g * P:(g + 1) * P, :])
            xsq = scr.tile([P, D], dtype=f32)
            for g in range(4):
                nc.scalar.activation(out=xsq[:], in_=xin[:, g * D:(g + 1) * D],
                                     func=AF.Square,
                                     accum_out=xss[:, g:g + 1])
                xsq = scr.tile([P, D], dtype=f32)
            xrs = xp.tile([P, 4], dtype=f32)      # 1/sumsq
            xrn = xp.tile([P, 4], dtype=f32)      # 1/norm
            nc.vector.reciprocal(out=xrs[:], in_=xss[:])
            nc.scalar.sqrt(out=xrn[:], in_=xrs[:])

            xn = xp.tile([P, 4 * D], dtype=f32)    # normalized X, f32
            xnb = xp.tile([P, 4 * D], dtype=bf16)  # normalized X, bf16
            for g in range(4):
                nc.vector.tensor_scalar_mul(xn[:, g * D:(g + 1) * D],
                                            xin[:, g * D:(g + 1) * D],
                                            xrn[:, g:g + 1])
            nc.vector.tensor_copy(xnb[:], xn[:])

            # XT: [d(part), k-major: k*512 + n] bf16
            xt = xp.tile([P, 4 * N], dtype=bf16)
            for k in range(4):
                pk = ptr_p.tile([P, 4 * P], dtype=bf16, space="PSUM", tag="tp")
                for g in range(4):
                    nc.tensor.transpose(
                        out=pk[:, g * P:(g + 1) * P],
                        in_=xnb[:, g * D + k * P: g * D + (k + 1) * P],
                        identity=ident[:])
                nc.vector.tensor_copy(out=xt[:, k * N:(k + 1) * N], in_=pk[:])

            # ---------------- sparse margin fixup (compute early) --------
            vals = None
            if fixup != "none":
                gidx = fix_p.tile([P, 4], dtype=i32)
                sidx = fix_p.tile([P, 4], dtype=i32)
                nc.sync.dma_start(out=gidx[:], in_=gidx_d.ap())
                nc.sync.dma_start(out=sidx[:], in_=sidx_d.ap())

                wg = fix_p.tile([P, 4 * D], dtype=f32)
                if fixup != "nogather":
                    for g in range(4):
                        nc.gpsimd.indirect_dma_start(
                            out=wg[:, g * D:(g + 1) * D], out_offset=None,
                            in_=w_d.ap(),
                            in_offset=bass.IndirectOffsetOnAxis(
                                ap=gidx[:, g:g + 1], axis=0))
                else:
                    nc.gpsimd.memset(wg[:], 1.0)

                st = fix_p.tile([P, 16], dtype=f32)   # stat columns
                sumsq = st[:, 0:4]
                for g in range(4):
                    wgsq = scr.tile([P, D], dtype=f32)
                    nc.scalar.activation(out=wgsq[:],
                                         in_=wg[:, g * D:(g + 1) * D],
                                         func=AF.Square,
                                         accum_out=sumsq[:, g:g + 1])
                rs = st[:, 4:8]
                rn = st[:, 8:12]
                nc.vector.reciprocal(out=rs[:], in_=sumsq[:])
                nc.scalar.sqrt(out=rn[:], in_=rs[:])       # 1/||w||

                dots = st[:, 12:16]
                for g in range(4):
                    dsc = scr.tile([P, D], dtype=f32)
                    nc.vector.tensor_tensor(
                        out=dsc[:], in0=xn[:, g * D:(g + 1) * D],
                        in1=wg[:, g * D:(g + 1) * D], op=A.mult)
                    nc.vector.tensor_reduce(
                        out=dots[:, g:g + 1], in_=dsc[:],
                        axis=mybir.AxisListType.X, op=A.add)

                ft = fix_p.tile([P, 4 * 8], dtype=f32)
                cosv, cos2, sine, phi, alt, _unused, fvals, tmp = (
                    ft[:, i * 4:(i + 1) * 4] for i in range(8))
                mask_t = fix_p.tile([P, 4], dtype=mybir.dt.uint8)
                mask = mask_t[:]
                nc.vector.tensor_tensor(out=cosv, in0=dots[:], in1=rn[:],
                                        op=A.mult)
                nc.vector.tensor_tensor(out=cos2, in0=cosv, in1=cosv,
                                        op=A.mult)
                nc.vector.tensor_scalar_min(cos2, cos2, 1.0)
                nc.scalar.activation(out=sine, in_=cos2, func=AF.Sqrt,
                                     scale=-1.0, bias=1.0)
                nc.vector.tensor_scalar_mul(phi, cosv, COS_M)
                nc.vector.tensor_scalar_mul(tmp, sine, SIN_M)
                nc.vector.tensor_tensor(out=phi, in0=phi, in1=tmp,
                                        op=A.subtract)
                nc.vector.tensor_scalar_add(alt, cosv, -MM)
                nc.vector.tensor_scalar(out=mask, in0=cosv, scalar1=TH,
                                        scalar2=None, op0=A.is_gt)
                nc.vector.select(out=fvals, mask=mask, on_true=phi,
                                 on_false=alt)
                nc.vector.tensor_scalar_mul(fvals, fvals, S)
                if out_bf16:
                    vals_t = fix_p.tile([P, 4], dtype=bf16)
                    nc.vector.tensor_copy(vals_t[:], fvals)
                    vals = vals_t[:]
                else:
                    vals = fvals

            # ---------------- W stream + matmul ----------------
            # bands of 4 chunks (512 classes); out groups of 4 bands (2048)
            w_chunked = w_d.ap().rearrange("(a p) d -> p a d", p=P)
            n_ogroups = (n_bands + 3) // 4
            ost = {}
            for b in range(n_bands):
                og = b // 4
                chunks = range(b * 4, min((b + 1) * 4, n_chunks))
                nsub = len(chunks)
                ncols = nsub * P

                if b % 4 == 0:
                    # allocate the out-staging tiles for this group
                    gbands = min(4, n_bands - og * 4)
                    gc = sum(
                        len(range(bb * 4, min((bb + 1) * 4, n_chunks))) * P
                        for bb in range(og * 4, og * 4 + gbands))
                    for n in range(4):
                        ost[n] = ob_p.tile([P, 4 * 512], dtype=out_dt,
                                           tag="ost", name=f"ost{og}_{n}")
                    ost_cols = gc

                # one 4-chunk (up to 1 MiB) W load
                wl = wl_p.tile([P, 4 * D], dtype=f32)
                nc.sync.dma_start(
                    out=wl[:].rearrange("p (a d) -> p a d", d=D)[:, :nsub, :],
                    in_=w_chunked[:, b * 4: b * 4 + nsub, :])
                # per-class norms
                wss = wst_p.tile([P, 4], dtype=f32, tag="wss")
                wrs = wst_p.tile([P, 4], dtype=f32, tag="wrs")
                wrn = wst_p.tile([P, 4], dtype=f32, tag="wrn")
                for s in range(nsub):
                    wsq = scr.tile([P, D], dtype=f32, tag="wsq")
                    nc.scalar.activation(out=wsq[:],
                                         in_=wl[:, s * D:(s + 1) * D],
                                         func=AF.Square,
                                         accum_out=wss[:, s:s + 1])
                nc.vector.reciprocal(out=wrs[:, :nsub], in_=wss[:, :nsub])
                nc.scalar.activation(out=wrn[:, :nsub], in_=wrs[:, :nsub],
                                     func=AF.Sqrt)

                wtb = wtb_p.tile([P, 4 * 512], dtype=bf16)
                for s0 in range(0, nsub, 2):
                    # two chunks per psum tile -> one batched drain
                    wtp = ptr_p.tile([P, 8 * P], dtype=bf16, space="PSUM",
                                     tag="tp")
                    for ds in range(2):
                        s = s0 + ds
                        # fused normalize + cast to bf16
                        wnb = wnb_p.tile([P, D], dtype=bf16, tag="wnb")
                        nc.vector.tensor_scalar_mul(wnb[:],
                                                    wl[:, s * D:(s + 1) * D],
                                                    wrn[:, s:s + 1])
                        for k in range(4):
                            nc.tensor.transpose(
                                out=wtp[:, k * 2 * P + ds * P:
                                        k * 2 * P + (ds + 1) * P],
                                in_=wnb[:, k * P:(k + 1) * P],
                                identity=ident[:])
                    # drain psum -> band tile (k-major layout)
                    nc.vector.tensor_copy(
                        out=wtb[:].rearrange("p (k c) -> p k c", k=4)
                            [:, :, s0 * P:(s0 + 2) * P],
                        in_=wtp[:].rearrange("p (k c) -> p k c", k=4))

                boff = (b - og * 4) * 512
                for n in range(4):
                    pm = pmm_p.tile([P, 512], dtype=f32, space="PSUM")
                    for k in range(4):
                        nc.tensor.matmul(
                            out=pm[:, :ncols],
                            lhsT=xt[:, k * N + n * P: k * N + (n + 1) * P],
                            rhs=wtb[:, k * 512: k * 512 + ncols],
                            start=(k == 0), stop=(k == 3))
                    if n < 2:
                        nc.scalar.mul(out=ost[n][:, boff:boff + ncols],
                                      in_=pm[:, :ncols], mul=S)
                    else:
                        nc.vector.tensor_scalar_mul(
                            ost[n][:, boff:boff + ncols], pm[:, :ncols], S)

                if b == n_bands - 1 or b % 4 == 3:
                    # flush the out-staging group (ACT-issued HWDGE)
                    for n in range(4):
                        nc.scalar.dma_start(
                            out=out2d[n * P:(n + 1) * P,
                                      og * 2048: og * 2048 + ost_cols],
                            in_=ost[n][:, :ost_cols])

            # ---------------- scatter the margin values ----------------
            if fixup not in ("none", "noscatter"):
                # all dense writes must land before the scatter
                tc.strict_bb_all_engine_barrier()
                for g in range(4):
                    nc.gpsimd.indirect_dma_start(
                        out=out_d.ap()[:, None],
                        out_offset=bass.IndirectOffsetOnAxis(
                            ap=sidx[:, g:g + 1], axis=0),
                        in_=vals[:, g:g + 1], in_offset=None)

    nc.compile()
    return nc


def make_in_maps(input, label, weight, n_chunks=98, c_per=C_PER):
    """Shard the full inputs into per-core input maps."""
    c_pad = n_chunks * P
    x = np.ascontiguousarray(input, dtype=np.float32)
    lab = np.asarray(label).astype(np.int64)
    w = np.asarray(weight, dtype=np.float32)
    rows = np.arange(N, dtype=np.int64)
    in_maps = []
    for i in range(N_CORES):
        c0 = i * c_per
        wi = np.empty((c_pad, D), dtype=np.float32)
        wi[:c_per] = w[c0:c0 + c_per]
        wi[c_per:] = 1.0
        loc = lab - c0
        valid = (loc >= 0) & (loc < c_per)
        g_rows = np.where(valid, loc, 0).astype(np.int32)
        cols = np.where(valid, loc, np.int64(c_pad - 1))
        s_flat = (rows * c_pad + cols).astype(np.int32)
        in_maps.append({
            "x": x,
            "w": wi,
            "gidx": np.ascontiguousarray(g_rows.reshape(4, P).T),
            "sidx": np.ascontiguousarray(s_flat.reshape(4, P).T),
        })
    return in_maps


def kernel(input, label, weight):
    """Full inputs in, full output out. Runs SPMD on 8 NeuronCores."""
    _ensure_ntff_hook()
    from concourse.bass_utils import run_bass_kernel_spmd

    if "nc" not in _cache:
        _cache["nc"] = build_nc()
    nc = _cache["nc"]

    in_maps = make_in_maps(input, label, weight)
    res = run_bass_kernel_spmd(nc, in_maps, list(range(N_CORES)))
    _cache["last_result"] = res

    c_pad = 98 * P
    out = np.concatenate(
        [res.results[i]["out"].reshape(N, c_pad)[:, :C_PER]
         for i in range(N_CORES)], axis=1)
    return out.astype(np.float32)


# revision 35
# speedup vs baseline: 1.3236x; 1.0201x over previous
"""ArcMarginProduct distributed Trainium2 kernel (8 NeuronCores).

Strategy (classifier/tensor parallel along out_features, per sharding hint):
  - weight [100000, 512] is row-sharded across 8 cores: 12500 classes each,
    padded to 12544 = 98*128 rows (pad rows are 1.0, outputs discarded).
  - input [512, 512] and label [512] are replicated (label passed as
    precomputed per-core local index tensors).
  - Each core computes out_i = S * cos(norm(X), norm(W_i)) for its class
    shard, plus the one-hot ArcFace margin applied at (n, label[n]) for the
    labels that fall in its shard (gather W rows -> phi -> scatter fixup).
  - Host concatenates the 8 [512, 12500] column blocks.

Device pipeline per core:
  X: load, row sumsq (ACT Square+accum), rsqrt (DVE recip + ACT sqrt),
     row-scale (f32 + bf16 copies), PE-transpose -> XT (bf16, d on partitions)
  W: stream 98 chunks of [128 rows, 512]: sumsq -> rsqrt -> normalize+cast
     bf16 -> 4x PE-transpose -> drain into per-band WT tiles (bf16)
  MM: per band of 512 classes: 4 n-chunks x 4 k-chunks bf16 matmuls,
      PSUM drain with x30 scale (ACT), DMA out.
  Fixup: indirect-gather W[label] rows, normalize, row-dot vs Xn (f32),
      phi = cos*cos(m) - sin(1-cos^2)*sin(m) with threshold select, x30,
      indirect-scatter 512 values into out (after a barrier).
"""

import math
import sys
import types

import numpy as np

# ---------------- constants (must match reference.py) ----------------
S = 30.0
M = 0.5
COS_M = math.cos(M)
SIN_M = math.sin(M)
TH = math.cos(math.pi - M)
MM = math.sin(math.pi - M) * M

N = 512          # batch
D = 512          # feature dim
C = 100000       # classes
N_CORES = 8
C_PER = C // N_CORES          # 12500
P = 128

_cache = {}


def _ensure_ntff_hook():
    """Install the axon NTFF profiling hook plumbing if this image's antenv
    lacks it (lets run_bass_kernel_spmd(trace=True) return exec_time_ns)."""
    try:
        import antenv.axon_hooks  # noqa: F401
        return
    except ImportError:
        pass
    import antenv
    m = types.ModuleType("antenv.axon_hooks")
    _hook = [None]
    m.set_axon_ntff_profile_hook = lambda h: _hook.__setitem__(0, h)
    m.get_axon_ntff_profile_hook = lambda: _hook[0]
    sys.modules["antenv.axon_hooks"] = m
    antenv.axon_hooks = m
    try:
        from trn_agent_boot.trn_boot import _ntff_profile_via_ctypes
        m.set_axon_ntff_profile_hook(
            _ntff_profile_via_ctypes("/opt/axon/libaxon_pjrt.so"))
    except Exception:
        pass


def build_nc(n_chunks=98, c_per=C_PER, fixup="full", out_bf16=True):
    """Build the per-core Bass graph. n_chunks*128 = padded shard width.

    fixup: "full" | "noscatter" | "nogather" | "none"
    """
    from contextlib import ExitStack

    import concourse.bass as bass
    import concourse.tile as tile
    from concourse import bacc, mybir
    from concourse.masks import make_identity

    f32 = mybir.dt.float32
    bf16 = mybir.dt.bfloat16
    i32 = mybir.dt.int32
    A = mybir.AluOpType
    AF = mybir.ActivationFunctionType

    c_pad = n_chunks * P
    n_bands = (n_chunks + 3) // 4        # bands of up to 4 chunks (512 cols)

    nc = bacc.Bacc("TRN2", target_bir_lowering=False, debug=False,
                   num_devices=N_CORES)

    x_d = nc.dram_tensor("x", [N, D], f32, kind="ExternalInput")
    w_d = nc.dram_tensor("w", [c_pad, D], f32, kind="ExternalInput")
    gidx_d = nc.dram_tensor("gidx", [P, 4], i32, kind="ExternalInput")
    sidx_d = nc.dram_tensor("sidx", [P, 4], i32, kind="ExternalInput")
    out_dt = bf16 if out_bf16 else f32
    out_d = nc.dram_tensor("out", [N * c_pad], out_dt, kind="ExternalOutput")

    out2d = out_d.ap().rearrange("(n c) -> n c", c=c_pad)

    with tile.TileContext(nc) as tc:
        with ExitStack() as ctx:
            const_p = ctx.enter_context(tc.tile_pool(name="const", bufs=1))
            xp = ctx.enter_context(tc.tile_pool(name="xp", bufs=1))
            scr = ctx.enter_context(tc.tile_pool(name="scr", bufs=2))
            wl_p = ctx.enter_context(tc.tile_pool(name="wl", bufs=5))
            wnb_p = ctx.enter_context(tc.tile_pool(name="wnb", bufs=6))
            wst_p = ctx.enter_context(tc.tile_pool(name="wst", bufs=5))
            wtb_p = ctx.enter_context(tc.tile_pool(name="wtb", bufs=3))
            ob_p = ctx.enter_context(tc.tile_pool(name="ob", bufs=12))
            fix_p = ctx.enter_context(tc.tile_pool(name="fix", bufs=1))
            ptr_p = ctx.enter_context(
                tc.tile_pool(name="ptr", bufs=3, space="PSUM"))
            pmm_p = ctx.enter_context(
                tc.tile_pool(name="pmm", bufs=5, space="PSUM"))

            ident = const_p.tile([P, P], dtype=bf16)
            make_identity(nc, ident[:])
            ones1 = const_p.tile([1, P], dtype=bf16)
            nc.gpsimd.memset(ones1[:], 1.0)

            # ---------------- W band stage 1 (load + norms) -------------
            w_chunked = w_d.ap().rearrange("(a p) d -> p a d", p=P)

            def band_stage1(b):
                nsub = min((b + 1) * 4, n_chunks) - b * 4
                wl = wl_p.tile([P, 4 * D], dtype=f32, tag="wl",
                               name=f"wl{b}")
                nc.sync.dma_start(
                    out=wl[:].rearrange("p (a d) -> p a d", d=D)[:, :nsub, :],
                    in_=w_chunked[:, b * 4: b * 4 + nsub, :])
                wss = wst_p.tile([P, 4], dtype=f32, tag="wss",
                                 name=f"wss{b}")
                wrs = wst_p.tile([P, 4], dtype=f32, tag="wrs",
                                 name=f"wrs{b}")
                wrn = wst_p.tile([P, 4], dtype=f32, tag="wrn",
                                 name=f"wrn{b}")
                for s in range(nsub):
                    wsq = scr.tile([P, D], dtype=f32, tag="wsq",
                                   name=f"wsq{b}_{s}")
                    nc.scalar.activation(out=wsq[:],
                                         in_=wl[:, s * D:(s + 1) * D],
                                         func=AF.Square,
                                         accum_out=wss[:, s:s + 1])
                nc.vector.reciprocal(out=wrs[:, :nsub], in_=wss[:, :nsub])
                nc.scalar.activation(out=wrn[:, :nsub], in_=wrs[:, :nsub],
                                     func=AF.Sqrt)
                return wl, wrn, nsub

            prepped = {}
            for b in range(min(3, n_bands)):
                prepped[b] = band_stage1(b)

            # ---------------- X preparation ----------------
            xin = xp.tile([P, 4 * D], dtype=f32)    # chunk g at cols g*512
            xss = xp.tile([P, 4], dtype=f32)
            for g in range(4):
                nc.sync.dma_start(out=xin[:, g * D:(g + 1) * D],
                                  in_=x_d.ap()[g * P:(g + 1) * P, :])
            for g in range(4):
                xsq = scr.tile([P, D], dtype=f32, tag="xsq",
                               name=f"xsq{g}")
                nc.vector.tensor_tensor(out=xsq[:],
                                        in0=xin[:, g * D:(g + 1) * D],
                                        in1=xin[:, g * D:(g + 1) * D],
                                        op=A.mult)
                nc.vector.tensor_reduce(out=xss[:, g:g + 1], in_=xsq[:],
                                        axis=mybir.AxisListType.X, op=A.add)
            xrs = xp.tile([P, 4], dtype=f32)      # 1/sumsq
            xrn = xp.tile([P, 4], dtype=f32)      # 1/norm
            nc.vector.reciprocal(out=xrs[:], in_=xss[:])
            nc.scalar.sqrt(out=xrn[:], in_=xrs[:])

            xn = xp.tile([P, 4 * D], dtype=f32)    # normalized X, f32
            xnb = xp.tile([P, 4 * D], dtype=bf16)  # normalized X, bf16
            for g in range(4):
                nc.vector.tensor_scalar_mul(xn[:, g * D:(g + 1) * D],
                                            xin[:, g * D:(g + 1) * D],
                                            xrn[:, g:g + 1])
            nc.vector.tensor_copy(xnb[:], xn[:])

            # XT: [d(part), k-major: k*512 + n] bf16
            xt = xp.tile([P, 4 * N], dtype=bf16)
            for k in range(4):
                pk = ptr_p.tile([P, 4 * P], dtype=bf16, space="PSUM", tag="tp")
                for g in range(4):
                    nc.tensor.transpose(
                        out=pk[:, g * P:(g + 1) * P],
                        in_=xnb[:, g * D + k * P: g * D + (k + 1) * P],
                        identity=ident[:])
                nc.vector.tensor_copy(out=xt[:, k * N:(k + 1) * N], in_=pk[:])

            # ---------------- sparse margin fixup (emitted mid-stream) ---
            fixst = {"vals": None, "sidx": None}

            def emit_fixup():
                gidx = fix_p.tile([P, 4], dtype=i32)
                sidx = fix_p.tile([P, 4], dtype=i32)
                nc.sync.dma_start(out=gidx[:], in_=gidx_d.ap())
                nc.sync.dma_start(out=sidx[:], in_=sidx_d.ap())

                wg = fix_p.tile([P, 4 * D], dtype=f32)
                if fixup != "nogather":
                    for g in range(4):
                        nc.gpsimd.indirect_dma_start(
                            out=wg[:, g * D:(g + 1) * D], out_offset=None,
                            in_=w_d.ap(),
                            in_offset=bass.IndirectOffsetOnAxis(
                                ap=gidx[:, g:g + 1], axis=0))
                else:
                    nc.gpsimd.memset(wg[:], 1.0)

                st = fix_p.tile([P, 16], dtype=f32)   # stat columns
                sumsq = st[:, 0:4]
                for g in range(4):
                    wgsq = scr.tile([P, D], dtype=f32)
                    nc.scalar.activation(out=wgsq[:],
                                         in_=wg[:, g * D:(g + 1) * D],
                                         func=AF.Square,
                                         accum_out=sumsq[:, g:g + 1])
                rs = st[:, 4:8]
                rn = st[:, 8:12]
                nc.vector.reciprocal(out=rs[:], in_=sumsq[:])
                nc.scalar.sqrt(out=rn[:], in_=rs[:])       # 1/||w||

                dots = st[:, 12:16]
                for g in range(4):
                    dsc = scr.tile([P, D], dtype=f32)
                    nc.vector.tensor_tensor(
                        out=dsc[:], in0=xn[:, g * D:(g + 1) * D],
                        in1=wg[:, g * D:(g + 1) * D], op=A.mult)
                    nc.vector.tensor_reduce(
                        out=dots[:, g:g + 1], in_=dsc[:],
                        axis=mybir.AxisListType.X, op=A.add)

                ft = fix_p.tile([P, 4 * 8], dtype=f32)
                cosv, cos2, sine, phi, alt, _unused, fvals, tmp = (
                    ft[:, i * 4:(i + 1) * 4] for i in range(8))
                mask_t = fix_p.tile([P, 4], dtype=mybir.dt.uint8)
                mask = mask_t[:]
                nc.vector.tensor_tensor(out=cosv, in0=dots[:], in1=rn[:],
                                        op=A.mult)
                nc.vector.tensor_tensor(out=cos2, in0=cosv, in1=cosv,
                                        op=A.mult)
                nc.vector.tensor_scalar_min(cos2, cos2, 1.0)
                nc.scalar.activation(out=sine, in_=cos2, func=AF.Sqrt,
                                     scale=-1.0, bias=1.0)
                nc.vector.tensor_scalar_mul(phi, cosv, COS_M)
                nc.vector.tensor_scalar_mul(tmp, sine, SIN_M)
                nc.vector.tensor_tensor(out=phi, in0=phi, in1=tmp,
                                        op=A.subtract)
                nc.vector.tensor_scalar_add(alt, cosv, -MM)
                nc.vector.tensor_scalar(out=mask, in0=cosv, scalar1=TH,
                                        scalar2=None, op0=A.is_gt)
                nc.vector.select(out=fvals, mask=mask, on_true=phi,
                                 on_false=alt)
                nc.vector.tensor_scalar_mul(fvals, fvals, S)
                if out_bf16:
                    vals_t = fix_p.tile([P, 4], dtype=bf16)
                    nc.vector.tensor_copy(vals_t[:], fvals)
                    fixst["vals"] = vals_t[:]
                else:
                    fixst["vals"] = fvals
                fixst["sidx"] = sidx

            # ---------------- W stream + matmul ----------------
            # bands of 4 chunks (512 classes); out groups of 4 bands (2048)
            n_ogroups = (n_bands + 3) // 4
            ost = {}
            for b in range(n_bands):
                og = b // 4
                chunks = range(b * 4, min((b + 1) * 4, n_chunks))
                nsub = len(chunks)
                ncols = nsub * P

                if b % 4 == 0:
                    # allocate the out-staging tiles for this group
                    gbands = min(4, n_bands - og * 4)
                    gc = sum(
                        len(range(bb * 4, min((bb + 1) * 4, n_chunks))) * P
                        for bb in range(og * 4, og * 4 + gbands))
                    for n in range(4):
                        ost[n] = ob_p.tile([P, 4 * 512], dtype=out_dt,
                                           tag="ost", name=f"ost{og}_{n}")
                    ost_cols = gc

                if b in prepped:
                    wl, wrn, _ = prepped.pop(b)
                else:
                    wl, wrn, _ = band_stage1(b)
                nxt = b + 3
                if nxt < n_bands and nxt not in prepped and nxt > 2:
                    prepped[nxt] = band_stage1(nxt)
                if b == 8 and fixup != "none":
                    emit_fixup()

                wtb = wtb_p.tile([P, 4 * 512], dtype=bf16)
                for s0 in range(0, nsub, 2):
                    # two chunks per psum tile -> one batched drain
                    wtp = ptr_p.tile([P, 8 * P], dtype=bf16, space="PSUM",
                                     tag="tp")
                    for ds in range(2):
                        s = s0 + ds
                        # fused normalize + cast to bf16
                        wnb = wnb_p.tile([P, D], dtype=bf16, tag="wnb")
                        nc.vector.tensor_scalar_mul(wnb[:],
                                                    wl[:, s * D:(s + 1) * D],
                                                    wrn[:, s:s + 1])
                        for k in range(4):
                            nc.tensor.transpose(
                                out=wtp[:, k * 2 * P + ds * P:
                                        k * 2 * P + (ds + 1) * P],
                                in_=wnb[:, k * P:(k + 1) * P],
                                identity=ident[:])
                    # drain psum -> band tile (k-major layout)
                    nc.vector.tensor_copy(
                        out=wtb[:].rearrange("p (k c) -> p k c", k=4)
                            [:, :, s0 * P:(s0 + 2) * P],
                        in_=wtp[:].rearrange("p (k c) -> p k c", k=4))

                boff = (b - og * 4) * 512
                for n in range(4):
                    pm = pmm_p.tile([P, 512], dtype=f32, space="PSUM")
                    for k in range(4):
                        nc.tensor.matmul(
                            out=pm[:, :ncols],
                            lhsT=xt[:, k * N + n * P: k * N + (n + 1) * P],
                            rhs=wtb[:, k * 512: k * 512 + ncols],
                            start=(k == 0), stop=(k == 3))
                    if n < 2:
                        nc.scalar.mul(out=ost[n][:, boff:boff + ncols],
                                      in_=pm[:, :ncols], mul=S)
                    else:
                        nc.vector.tensor_scalar_mul(
                            ost[n][:, boff:boff + ncols], pm[:, :ncols], S)

                if b == n_bands - 1 or b % 4 == 3:
                    # flush the out-staging group
                    for n in range(4):
                        nc.sync.dma_start(
                            out=out2d[n * P:(n + 1) * P,
                                      og * 2048: og * 2048 + ost_cols],
                            in_=ost[n][:, :ost_cols])

            # ---------------- scatter the margin values ----------------
            if fixup != "none" and fixst["vals"] is None:
                emit_fixup()         # tiny configs never reach band 8
            if fixup not in ("none", "noscatter"):
                vals, sidx = fixst["vals"], fixst["sidx"]
                # all dense writes must land before the scatter
                tc.strict_bb_all_engine_barrier()
                for g in range(4):
                    nc.gpsimd.indirect_dma_start(
                        out=out_d.ap()[:, None],
                        out_offset=bass.IndirectOffsetOnAxis(
                            ap=sidx[:, g:g + 1], axis=0),
                        in_=vals[:, g:g + 1], in_offset=None)

    nc.compile()
    return nc


def make_in_maps(input, label, weight, n_chunks=98, c_per=C_PER):
    """Shard the full inputs into per-core input maps."""
    c_pad = n_chunks * P
    x = np.ascontiguousarray(input, dtype=np.float32)
    lab = np.asarray(label).astype(np.int64)
    w = np.asarray(weight, dtype=np.float32)
    rows = np.arange(N, dtype=np.int64)
    in_maps = []
    for i in range(N_CORES):
        c0 = i * c_per
        wi = np.empty((c_pad, D), dtype=np.float32)
        wi[:c_per] = w[c0:c0 + c_per]
        wi[c_per:] = 1.0
        loc = lab - c0
        valid = (loc >= 0) & (loc < c_per)
        g_rows = np.where(valid, loc, 0).astype(np.int32)
        cols = np.where(valid, loc, np.int64(c_pad - 1))
        s_flat = (rows * c_pad + cols).astype(np.int32)
        in_maps.append({
            "x": x,
            "w": wi,
            "gidx": np.ascontiguousarray(g_rows.reshape(4, P).T),
            "sidx": np.ascontiguousarray(s_flat.reshape(4, P).T),
        })
    return in_maps


def kernel(input, label, weight):
    """Full inputs in, full output out. Runs SPMD on 8 NeuronCores."""
    _ensure_ntff_hook()
    from concourse.bass_utils import run_bass_kernel_spmd

    if "nc" not in _cache:
        _cache["nc"] = build_nc()
    nc = _cache["nc"]

    in_maps = make_in_maps(input, label, weight)
    res = run_bass_kernel_spmd(nc, in_maps, list(range(N_CORES)))
    _cache["last_result"] = res

    c_pad = 98 * P
    out = np.concatenate(
        [res.results[i]["out"].reshape(N, c_pad)[:, :C_PER]
         for i in range(N_CORES)], axis=1)
    return out.astype(np.float32)


# revision 36
# speedup vs baseline: 1.3635x; 1.0301x over previous
"""ArcMarginProduct distributed Trainium2 kernel (8 NeuronCores).

Strategy (classifier/tensor parallel along out_features, per sharding hint):
  - weight [100000, 512] is row-sharded across 8 cores: 12500 classes each,
    padded to 12544 = 98*128 rows (pad rows are 1.0, outputs discarded).
  - input [512, 512] and label [512] are replicated (label passed as
    precomputed per-core local index tensors).
  - Each core computes out_i = S * cos(norm(X), norm(W_i)) for its class
    shard, plus the one-hot ArcFace margin applied at (n, label[n]) for the
    labels that fall in its shard (gather W rows -> phi -> scatter fixup).
  - Host concatenates the 8 [512, 12500] column blocks.

Device pipeline per core:
  X: load, row sumsq (ACT Square+accum), rsqrt (DVE recip + ACT sqrt),
     row-scale (f32 + bf16 copies), PE-transpose -> XT (bf16, d on partitions)
  W: stream 98 chunks of [128 rows, 512]: sumsq -> rsqrt -> normalize+cast
     bf16 -> 4x PE-transpose -> drain into per-band WT tiles (bf16)
  MM: per band of 512 classes: 4 n-chunks x 4 k-chunks bf16 matmuls,
      PSUM drain with x30 scale (ACT), DMA out.
  Fixup: indirect-gather W[label] rows, normalize, row-dot vs Xn (f32),
      phi = cos*cos(m) - sin(1-cos^2)*sin(m) with threshold select, x30,
      indirect-scatter 512 values into out (after a barrier).
"""

import math
import sys
import types

import numpy as np

# ---------------- constants (must match reference.py) ----------------
S = 30.0
M = 0.5
COS_M = math.cos(M)
SIN_M = math.sin(M)
TH = math.cos(math.pi - M)
MM = math.sin(math.pi - M) * M

N = 512          # batch
D = 512          # feature dim
C = 100000       # classes
N_CORES = 8
C_PER = C // N_CORES          # 12500
P = 128

_cache = {}


def _ensure_ntff_hook():
    """Install the axon NTFF profiling hook plumbing if this image's antenv
    lacks it (lets run_bass_kernel_spmd(trace=True) return exec_time_ns)."""
    try:
        import antenv.axon_hooks  # noqa: F401
        return
    except ImportError:
        pass
    import antenv
    m = types.ModuleType("antenv.axon_hooks")
    _hook = [None]
    m.set_axon_ntff_profile_hook = lambda h: _hook.__setitem__(0, h)
    m.get_axon_ntff_profile_hook = lambda: _hook[0]
    sys.modules["antenv.axon_hooks"] = m
    antenv.axon_hooks = m
    try:
        from trn_agent_boot.trn_boot import _ntff_profile_via_ctypes
        m.set_axon_ntff_profile_hook(
            _ntff_profile_via_ctypes("/opt/axon/libaxon_pjrt.so"))
    except Exception:
        pass


def build_nc(n_chunks=98, c_per=C_PER, fixup="full", out_bf16=True):
    """Build the per-core Bass graph. n_chunks*128 = padded shard width.

    fixup: "full" | "noscatter" | "nogather" | "none"
    """
    from contextlib import ExitStack

    import concourse.bass as bass
    import concourse.tile as tile
    from concourse import bacc, mybir
    from concourse.masks import make_identity

    f32 = mybir.dt.float32
    bf16 = mybir.dt.bfloat16
    i32 = mybir.dt.int32
    A = mybir.AluOpType
    AF = mybir.ActivationFunctionType

    c_pad = n_chunks * P
    n_bands = (n_chunks + 3) // 4        # bands of up to 4 chunks (512 cols)

    nc = bacc.Bacc("TRN2", target_bir_lowering=False, debug=False,
                   num_devices=N_CORES)

    x_d = nc.dram_tensor("x", [N, D], f32, kind="ExternalInput")
    w_d = nc.dram_tensor("w", [c_pad, D], f32, kind="ExternalInput")
    gidx_d = nc.dram_tensor("gidx", [P, 4], i32, kind="ExternalInput")
    sidx_d = nc.dram_tensor("sidx", [P, 4], i32, kind="ExternalInput")
    out_dt = bf16 if out_bf16 else f32
    out_d = nc.dram_tensor("out", [N * c_pad], out_dt, kind="ExternalOutput")

    out2d = out_d.ap().rearrange("(n c) -> n c", c=c_pad)

    with tile.TileContext(nc) as tc:
        with ExitStack() as ctx:
            const_p = ctx.enter_context(tc.tile_pool(name="const", bufs=1))
            xp = ctx.enter_context(tc.tile_pool(name="xp", bufs=1))
            scr = ctx.enter_context(tc.tile_pool(name="scr", bufs=2))
            wl_p = ctx.enter_context(tc.tile_pool(name="wl", bufs=5))
            wnb_p = ctx.enter_context(tc.tile_pool(name="wnb", bufs=6))
            wst_p = ctx.enter_context(tc.tile_pool(name="wst", bufs=5))
            wtb_p = ctx.enter_context(tc.tile_pool(name="wtb", bufs=3))
            ob_p = ctx.enter_context(tc.tile_pool(name="ob", bufs=12))
            fix_p = ctx.enter_context(tc.tile_pool(name="fix", bufs=1))
            ptr_p = ctx.enter_context(
                tc.tile_pool(name="ptr", bufs=3, space="PSUM"))
            pmm_p = ctx.enter_context(
                tc.tile_pool(name="pmm", bufs=5, space="PSUM"))

            ident = const_p.tile([P, P], dtype=bf16)
            make_identity(nc, ident[:])
            ones1 = const_p.tile([1, P], dtype=bf16)
            nc.gpsimd.memset(ones1[:], 1.0)

            # ---------------- W band stage 1 (load + norms) -------------
            w_chunked = w_d.ap().rearrange("(a p) d -> p a d", p=P)

            def band_stage1(b):
                nsub = min((b + 1) * 4, n_chunks) - b * 4
                wl = wl_p.tile([P, 4 * D], dtype=f32, tag="wl",
                               name=f"wl{b}")
                nc.sync.dma_start(
                    out=wl[:].rearrange("p (a d) -> p a d", d=D)[:, :nsub, :],
                    in_=w_chunked[:, b * 4: b * 4 + nsub, :])
                wss = wst_p.tile([P, 4], dtype=f32, tag="wss",
                                 name=f"wss{b}")
                wrs = wst_p.tile([P, 4], dtype=f32, tag="wrs",
                                 name=f"wrs{b}")
                wrn = wst_p.tile([P, 4], dtype=f32, tag="wrn",
                                 name=f"wrn{b}")
                for s in range(nsub):
                    wsq = scr.tile([P, D], dtype=f32, tag="wsq",
                                   name=f"wsq{b}_{s}")
                    nc.scalar.activation(out=wsq[:],
                                         in_=wl[:, s * D:(s + 1) * D],
                                         func=AF.Square,
                                         accum_out=wss[:, s:s + 1])
                nc.vector.reciprocal(out=wrs[:, :nsub], in_=wss[:, :nsub])
                nc.scalar.activation(out=wrn[:, :nsub], in_=wrs[:, :nsub],
                                     func=AF.Sqrt)
                return wl, wrn, nsub

            # ---------------- X preparation ----------------
            xin = xp.tile([P, 4 * D], dtype=f32)    # chunk g at cols g*512
            xss = xp.tile([P, 4], dtype=f32)
            for g in range(4):
                nc.sync.dma_start(out=xin[:, g * D:(g + 1) * D],
                                  in_=x_d.ap()[g * P:(g + 1) * P, :])
            for g in range(4):
                xsq = scr.tile([P, D], dtype=f32, tag="xsq",
                               name=f"xsq{g}")
                nc.vector.tensor_tensor(out=xsq[:],
                                        in0=xin[:, g * D:(g + 1) * D],
                                        in1=xin[:, g * D:(g + 1) * D],
                                        op=A.mult)
                nc.vector.tensor_reduce(out=xss[:, g:g + 1], in_=xsq[:],
                                        axis=mybir.AxisListType.X, op=A.add)
            xrs = xp.tile([P, 4], dtype=f32)      # 1/sumsq
            xrn = xp.tile([P, 4], dtype=f32)      # 1/norm
            nc.vector.reciprocal(out=xrs[:], in_=xss[:])
            nc.scalar.sqrt(out=xrn[:], in_=xrs[:])

            xn = xp.tile([P, 4 * D], dtype=f32)    # normalized X, f32
            xnb = xp.tile([P, 4 * D], dtype=bf16)  # normalized X, bf16
            for g in range(4):
                nc.vector.tensor_scalar_mul(xn[:, g * D:(g + 1) * D],
                                            xin[:, g * D:(g + 1) * D],
                                            xrn[:, g:g + 1])
            nc.vector.tensor_copy(xnb[:], xn[:])

            # XT: [d(part), k-major: k*512 + n] bf16
            xt = xp.tile([P, 4 * N], dtype=bf16)
            for k in range(4):
                pk = ptr_p.tile([P, 4 * P], dtype=bf16, space="PSUM", tag="tp")
                for g in range(4):
                    nc.tensor.transpose(
                        out=pk[:, g * P:(g + 1) * P],
                        in_=xnb[:, g * D + k * P: g * D + (k + 1) * P],
                        identity=ident[:])
                nc.vector.tensor_copy(out=xt[:, k * N:(k + 1) * N], in_=pk[:])

            prepped = {}
            for b in range(min(3, n_bands)):
                prepped[b] = band_stage1(b)

            # ---------------- sparse margin fixup (emitted mid-stream) ---
            fixst = {"vals": None, "sidx": None}

            def emit_fixup():
                gidx = fix_p.tile([P, 4], dtype=i32)
                sidx = fix_p.tile([P, 4], dtype=i32)
                nc.sync.dma_start(out=gidx[:], in_=gidx_d.ap())
                nc.sync.dma_start(out=sidx[:], in_=sidx_d.ap())

                wg = fix_p.tile([P, 4 * D], dtype=f32)
                if fixup != "nogather":
                    for g in range(4):
                        nc.gpsimd.indirect_dma_start(
                            out=wg[:, g * D:(g + 1) * D], out_offset=None,
                            in_=w_d.ap(),
                            in_offset=bass.IndirectOffsetOnAxis(
                                ap=gidx[:, g:g + 1], axis=0))
                else:
                    nc.gpsimd.memset(wg[:], 1.0)

                st = fix_p.tile([P, 16], dtype=f32)   # stat columns
                sumsq = st[:, 0:4]
                for g in range(4):
                    wgsq = scr.tile([P, D], dtype=f32)
                    nc.scalar.activation(out=wgsq[:],
                                         in_=wg[:, g * D:(g + 1) * D],
                                         func=AF.Square,
                                         accum_out=sumsq[:, g:g + 1])
                rs = st[:, 4:8]
                rn = st[:, 8:12]
                nc.vector.reciprocal(out=rs[:], in_=sumsq[:])
                nc.scalar.sqrt(out=rn[:], in_=rs[:])       # 1/||w||

                dots = st[:, 12:16]
                for g in range(4):
                    dsc = scr.tile([P, D], dtype=f32)
                    nc.vector.tensor_tensor(
                        out=dsc[:], in0=xn[:, g * D:(g + 1) * D],
                        in1=wg[:, g * D:(g + 1) * D], op=A.mult)
                    nc.vector.tensor_reduce(
                        out=dots[:, g:g + 1], in_=dsc[:],
                        axis=mybir.AxisListType.X, op=A.add)

                ft = fix_p.tile([P, 4 * 8], dtype=f32)
                cosv, cos2, sine, phi, alt, _unused, fvals, tmp = (
                    ft[:, i * 4:(i + 1) * 4] for i in range(8))
                mask_t = fix_p.tile([P, 4], dtype=mybir.dt.uint8)
                mask = mask_t[:]
                nc.vector.tensor_tensor(out=cosv, in0=dots[:], in1=rn[:],
                                        op=A.mult)
                nc.vector.tensor_tensor(out=cos2, in0=cosv, in1=cosv,
                                        op=A.mult)
                nc.vector.tensor_scalar_min(cos2, cos2, 1.0)
                nc.scalar.activation(out=sine, in_=cos2, func=AF.Sqrt,
                                     scale=-1.0, bias=1.0)
                nc.vector.tensor_scalar_mul(phi, cosv, COS_M)
                nc.vector.tensor_scalar_mul(tmp, sine, SIN_M)
                nc.vector.tensor_tensor(out=phi, in0=phi, in1=tmp,
                                        op=A.subtract)
                nc.vector.tensor_scalar_add(alt, cosv, -MM)
                nc.vector.tensor_scalar(out=mask, in0=cosv, scalar1=TH,
                                        scalar2=None, op0=A.is_gt)
                nc.vector.select(out=fvals, mask=mask, on_true=phi,
                                 on_false=alt)
                nc.vector.tensor_scalar_mul(fvals, fvals, S)
                if out_bf16:
                    vals_t = fix_p.tile([P, 4], dtype=bf16)
                    nc.vector.tensor_copy(vals_t[:], fvals)
                    fixst["vals"] = vals_t[:]
                else:
                    fixst["vals"] = fvals
                fixst["sidx"] = sidx

            # ---------------- W stream + matmul ----------------
            # bands of 4 chunks (512 classes); out groups of 4 bands (2048)
            n_ogroups = (n_bands + 3) // 4
            ost = {}
            for b in range(n_bands):
                og = b // 4
                chunks = range(b * 4, min((b + 1) * 4, n_chunks))
                nsub = len(chunks)
                ncols = nsub * P

                if b % 4 == 0:
                    # allocate the out-staging tiles for this group
                    gbands = min(4, n_bands - og * 4)
                    gc = sum(
                        len(range(bb * 4, min((bb + 1) * 4, n_chunks))) * P
                        for bb in range(og * 4, og * 4 + gbands))
                    for n in range(4):
                        ost[n] = ob_p.tile([P, 4 * 512], dtype=out_dt,
                                           tag="ost", name=f"ost{og}_{n}")
                    ost_cols = gc

                if b in prepped:
                    wl, wrn, _ = prepped.pop(b)
                else:
                    wl, wrn, _ = band_stage1(b)
                nxt = b + 3
                if nxt < n_bands and nxt not in prepped and nxt > 2:
                    prepped[nxt] = band_stage1(nxt)
                if b == 8 and fixup != "none":
                    emit_fixup()

                wtb = wtb_p.tile([P, 4 * 512], dtype=bf16)
                for s0 in range(0, nsub, 2):
                    # two chunks per psum tile -> one batched drain
                    wtp = ptr_p.tile([P, 8 * P], dtype=bf16, space="PSUM",
                                     tag="tp")
                    for ds in range(2):
                        s = s0 + ds
                        # fused normalize + cast to bf16
                        wnb = wnb_p.tile([P, D], dtype=bf16, tag="wnb")
                        nc.vector.tensor_scalar_mul(wnb[:],
                                                    wl[:, s * D:(s + 1) * D],
                                                    wrn[:, s:s + 1])
                        for k in range(4):
                            nc.tensor.transpose(
                                out=wtp[:, k * 2 * P + ds * P:
                                        k * 2 * P + (ds + 1) * P],
                                in_=wnb[:, k * P:(k + 1) * P],
                                identity=ident[:])
                    # drain psum -> band tile (k-major layout)
                    nc.vector.tensor_copy(
                        out=wtb[:].rearrange("p (k c) -> p k c", k=4)
                            [:, :, s0 * P:(s0 + 2) * P],
                        in_=wtp[:].rearrange("p (k c) -> p k c", k=4))

                boff = (b - og * 4) * 512
                for n in range(4):
                    pm = pmm_p.tile([P, 512], dtype=f32, space="PSUM")
                    for k in range(4):
                        nc.tensor.matmul(
                            out=pm[:, :ncols],
                            lhsT=xt[:, k * N + n * P: k * N + (n + 1) * P],
                            rhs=wtb[:, k * 512: k * 512 + ncols],
                            start=(k == 0), stop=(k == 3))
                    if n < 2:
                        nc.scalar.mul(out=ost[n][:, boff:boff + ncols],
                                      in_=pm[:, :ncols], mul=S)
                    else:
                        nc.vector.tensor_scalar_mul(
                            ost[n][:, boff:boff + ncols], pm[:, :ncols], S)

                if b == n_bands - 1 or b % 4 == 3:
                    # flush the out-staging group
                    for n in range(4):
                        nc.sync.dma_start(
                            out=out2d[n * P:(n + 1) * P,
                                      og * 2048: og * 2048 + ost_cols],
                            in_=ost[n][:, :ost_cols])

            # ---------------- scatter the margin values ----------------
            if fixup != "none" and fixst["vals"] is None:
                emit_fixup()         # tiny configs never reach band 8
            if fixup not in ("none", "noscatter"):
                vals, sidx = fixst["vals"], fixst["sidx"]
                # all dense writes must land before the scatter
                tc.strict_bb_all_engine_barrier()
                for g in range(4):
                    nc.gpsimd.indirect_dma_start(
                        out=out_d.ap()[:, None],
                        out_offset=bass.IndirectOffsetOnAxis(
                            ap=sidx[:, g:g + 1], axis=0),
                        in_=vals[:, g:g + 1], in_offset=None)

    nc.compile()
    return nc


def make_in_maps(input, label, weight, n_chunks=98, c_per=C_PER):
    """Shard the full inputs into per-core input maps."""
    c_pad = n_chunks * P
    x = np.ascontiguousarray(input, dtype=np.float32)
    lab = np.asarray(label).astype(np.int64)
    w = np.asarray(weight, dtype=np.float32)
    rows = np.arange(N, dtype=np.int64)
    in_maps = []
    for i in range(N_CORES):
        c0 = i * c_per
        wi = np.empty((c_pad, D), dtype=np.float32)
        wi[:c_per] = w[c0:c0 + c_per]
        wi[c_per:] = 1.0
        loc = lab - c0
        valid = (loc >= 0) & (loc < c_per)
        g_rows = np.where(valid, loc, 0).astype(np.int32)
        cols = np.where(valid, loc, np.int64(c_pad - 1))
        s_flat = (rows * c_pad + cols).astype(np.int32)
        in_maps.append({
            "x": x,
            "w": wi,
            "gidx": np.ascontiguousarray(g_rows.reshape(4, P).T),
            "sidx": np.ascontiguousarray(s_flat.reshape(4, P).T),
        })
    return in_maps


def kernel(input, label, weight):
    """Full inputs in, full output out. Runs SPMD on 8 NeuronCores."""
    _ensure_ntff_hook()
    from concourse.bass_utils import run_bass_kernel_spmd

    if "nc" not in _cache:
        _cache["nc"] = build_nc()
    nc = _cache["nc"]

    in_maps = make_in_maps(input, label, weight)
    res = run_bass_kernel_spmd(nc, in_maps, list(range(N_CORES)))
    _cache["last_result"] = res

    c_pad = 98 * P
    out = np.concatenate(
        [res.results[i]["out"].reshape(N, c_pad)[:, :C_PER]
         for i in range(N_CORES)], axis=1)
    return out.astype(np.float32)
